# revision 1
# baseline (speedup 1.0000x reference)
"""Trainium2 Bass kernel for nn_LocalInferenceModeling (cross-attention enhance).

Reference computation (per batch b):
    e = x1 @ x2^T                                  [L, L]
    a12 = softmax_j(e + m2[j]);  x1t = a12 @ x2    [L, H]
    a21 = softmax_i(e^T + m1[i]); x2t = a21 @ x1   [L, H]
    y1 = concat([x1, x1t, x1 - x1t, x1 * x1t], -1) [L, 4H]
    y2 = concat([x2, x2t, x2 - x2t, x2 * x2t], -1)

Sharding: batch dim B=32 split across 8 NeuronCores (4 batches/core),
no communication.

Device-side redesign vs the fp32 baseline:
  - Host supplies bf16 inputs, both natural ([L,H]) and pre-transposed
    ([H,L]); PE matmuls run bf16 (1 cyc/row), halving DMA bytes.
  - e is computed ONCE; e^T comes from 16 exact fp32 PE transposes of
    the e SBUF copy instead of a second 32-matmul pass.
  - Probs are produced directly in TRANSPOSED (contraction-ready)
    layout, so the baseline's 32 per-batch probs transposes vanish:
      p12T[j,i] = exp(e^T[j,i] - rowmax_i)   (pad-j rows self-masked)
      p21T[i,j] = exp(e[i,j] - colmax_j + m1col[i])
    Masking uses a bf16-exact sentinel (-29952) for m2/m1 so that pad
    rows stay recoverable (the sentinel shift cancels against the
    matching shift in the subtracted stabilizer); true -1e30 masking
    enters only via per-partition activation bias where it is exact.
    Stabilizer (max) values are applied via rank-1 ones (x) row matmuls;
    their bf16 rounding is uniform per output row/col and cancels in the
    z-normalization.
  - z = sum(exp) comes from tiny N=1 matmuls against a ones column
    (partition-dim sums), normalization is folded into the psum->SBUF
    copy on the Activation engine, enhance (sub/mul) runs all-bf16 on
    DVE at 2x, outputs are written bf16 (3H slice only); the host
    upcasts and prepends the x_bar slice.
  - DMAs are spread over the three legal issue queues (SP / Activation /
    GpSimd) since queue occupancy, not bus bytes, is the limiter.
"""

import sys

import numpy as np

sys.path.insert(0, "/opt/trn_rl_repo")

from contextlib import ExitStack

import ml_dtypes

import concourse.bass as bass
import concourse.bacc as bacc
import concourse.bass_isa as bass_isa
import concourse.mybir as mybir
from concourse import masks
from concourse.bass_utils import run_bass_kernel_spmd
from concourse.tile import TileContext

B, L, H = 32, 512, 1024
NCORES = 8
BPC = B // NCORES  # batches per core
NT = L // 128  # 4 partition tiles per L
HT = H // 128  # 8 partition tiles per H

SENT = np.float32(29952.0)  # bf16-exact sentinel magnitude
NEG = np.float32(-1.0e30)

F32 = mybir.dt.float32
F32R = mybir.dt.float32r
BF16 = mybir.dt.bfloat16
NPBF16 = np.dtype(ml_dtypes.bfloat16)

Exp = mybir.ActivationFunctionType.Exp
Copy = mybir.ActivationFunctionType.Copy
AX = mybir.AxisListType.X

_NC_CACHE = {}


def build_nc():
    nc = bacc.Bacc(None, target_bir_lowering=False)
    xb1 = nc.dram_tensor("xb1", [BPC, L, H], BF16, kind="ExternalInput")
    xb2 = nc.dram_tensor("xb2", [BPC, L, H], BF16, kind="ExternalInput")
    xt1 = nc.dram_tensor("xt1", [BPC, H, L], F32R, kind="ExternalInput")
    xt2 = nc.dram_tensor("xt2", [BPC, H, L], F32R, kind="ExternalInput")
    m2row = nc.dram_tensor("m2row", [BPC, L], BF16, kind="ExternalInput")
    m1rowneg = nc.dram_tensor("m1rowneg", [BPC, L], BF16, kind="ExternalInput")
    # partition-dim (column) masks, f32, pre-swizzled [128, BPC*NT]
    m1col = nc.dram_tensor("m1col", [128, BPC * NT], F32, kind="ExternalInput")
    m1colsent = nc.dram_tensor("m1colsent", [128, BPC * NT], F32, kind="ExternalInput")
    y1 = nc.dram_tensor("y1", [BPC, L, 3 * H], BF16, kind="ExternalOutput")
    y2 = nc.dram_tensor("y2", [BPC, L, 3 * H], BF16, kind="ExternalOutput")

    # DMA issue queues, round-robined
    dmaqs = [nc.sync, nc.scalar, nc.gpsimd]

    with TileContext(nc) as tc, ExitStack() as ctx:
        from concourse.tile import add_dep_helper

        const = ctx.enter_context(tc.tile_pool(name="const", bufs=1))
        ident = const.tile([128, 128], F32)
        masks.make_identity(nc, ident[:])
        onesb = const.tile([1, 128], BF16)
        nc.vector.memset(onesb[:], 1.0)
        onescol = const.tile([128, 1], BF16)
        nc.vector.memset(onescol[:], 1.0)
        ones32 = const.tile([1, 32], F32)
        nc.vector.memset(ones32[:], 1.0)
        ones32col = const.tile([128, 1], F32)
        nc.vector.memset(ones32col[:], 1.0)


        xp = ctx.enter_context(tc.tile_pool(name="xp", bufs=2))
        esb = ctx.enter_context(tc.tile_pool(name="esb", bufs=6))
        pp = ctx.enter_context(tc.tile_pool(name="pp", bufs=2 * NT))
        st = ctx.enter_context(tc.tile_pool(name="st", bufs=3))
        yp = ctx.enter_context(tc.tile_pool(name="yp", bufs=4))
        mrp = ctx.enter_context(tc.tile_pool(name="mrp", bufs=1))
        pmp = ctx.enter_context(tc.tile_pool(name="pmp", bufs=2))
        stp = ctx.enter_context(tc.tile_pool(name="stp", bufs=2))
        psE = ctx.enter_context(tc.tile_pool(name="psE", bufs=2, space="PSUM"))
        psT = ctx.enter_context(tc.tile_pool(name="psT", bufs=2, space="PSUM"))
        psB = ctx.enter_context(tc.tile_pool(name="psB", bufs=2, space="PSUM"))
        psS = ctx.enter_context(tc.tile_pool(name="psS", bufs=1, space="PSUM"))
        psScr = ctx.enter_context(tc.tile_pool(name="psScr", bufs=1, space="PSUM"))
        scratch = psScr.tile([32, 32], F32, name="scratch", tag="scratch")

        gates = {"psE": [], "psT": [], "psB": [], "psS": []}

        touch_cnt = [0]

        def touch(ap):
            # Tiny PE matmul reading `ap` so the PE engine observes the
            # producer's sem tick; real matmuls then carry at most one sync
            # wait. Rotate over scratch columns so touches don't WAW-chain
            # each other (a shared column would serialize all touches, and
            # with them the whole PE stream, into program order).
            p = min(ap.shape[0], 32)
            f = min(ap.shape[1], 32)
            if ap.dtype == F32R:
                ap = ap.bitcast(F32)
            oc = onescol if ap.dtype == BF16 else ones32col
            col = touch_cnt[0] % 32
            touch_cnt[0] += 1
            with tc.high_priority(offset=200):
                return nc.tensor.matmul(
                    scratch[0:f, col : col + 1], ap[0:p, 0:f], oc[0:p, 0:1],
                    start=True, stop=True)

        def gate(tag, bufs, first_inst):
            # Order the group's first PE write after the touch that observed
            # the release of the slot it reuses (bufs groups back).
            hist = gates[tag]
            k = len(hist)
            if k >= bufs and hist[k - bufs] is not None:
                add_dep_helper(first_inst.ins, hist[k - bufs].ins, sync=False,
                               reason="psum slot gate")
            hist.append(None)
            return k

        def set_gate(tag, k, tinst):
            gates[tag][k] = tinst

        touch(ident)
        nc.tensor.matmul(scratch[0:32, 0:1], ones32[0:1, :], ones32[0:1, 0:1],
                         start=True, stop=True)

        # ---- static mask loads ----
        m2r = mrp.tile([1, BPC * L], BF16, name="m2r", tag="m2r")
        m1rn = mrp.tile([1, BPC * L], BF16, name="m1rn", tag="m1rn")
        m1c = mrp.tile([128, BPC * NT], F32, name="m1c", tag="m1c")
        m1cs = mrp.tile([128, BPC * NT], F32, name="m1cs", tag="m1cs")
        # load order matters for batch 0: m1cs feeds the very first e_sb
        # adds, m2r the first rank-1; m2rn/m1rn are needed only later
        nc.scalar.dma_start(m1cs[:], m1colsent[:, :])
        nc.scalar.dma_start(m1c[:], m1col[:, :])
        nc.scalar.dma_start(m2r[:1, :], m2row.rearrange("b l -> (b l)")[None, :])
        nc.scalar.dma_start(m1rn[:1, :], m1rowneg.rearrange("b l -> (b l)")[None, :])
        # no touches for the mask rows: each rank-1 matmul consuming them has
        # a single unobserved producer, which its own sem wait covers

        def load_batch(b):
            xb1t = xp.tile([128, NT * H], BF16, name="xb1t", tag="xb1t")
            xb2t = xp.tile([128, NT * H], BF16, name="xb2t", tag="xb2t")
            xt1t = xp.tile([128, HT * L], F32R, name="xt1t", tag="xt1t")
            xt2t = xp.tile([128, HT * L], F32R, name="xt2t", tag="xt2t")
            # transposed operands first: the e matmuls only need these
            dmaqs[0].dma_start(
                xt1t[:].rearrange("p (c l) -> p c l", c=HT),
                xt1[b].rearrange("(c p) l -> p c l", p=128))
            dmaqs[2].dma_start(
                xt2t[:].rearrange("p (c l) -> p c l", c=HT),
                xt2[b].rearrange("(c p) l -> p c l", p=128))
            dmaqs[0].dma_start(
                xb1t[:].rearrange("p (a h) -> p a h", a=NT),
                xb1[b].rearrange("(a p) h -> p a h", p=128))
            dmaqs[2].dma_start(
                xb2t[:].rearrange("p (a h) -> p a h", a=NT),
                xb2[b].rearrange("(a p) h -> p a h", p=128))
            return xt1t, xt2t, xb1t, xb2t

        def emit_head(b, xt1t, xt2t):
            """e psum (raw + m2 sentinel), row stats, e_sb, nm1r."""
            touch(xt1t)
            touch(xt2t)
            m2row_b = m2r[0:1, L * b : L * (b + 1)]
            nm4 = st.tile([128, NT], F32, name="nm4", tag="nm4")
            e_sb = [esb.tile([128, L], F32, name="e_sb", tag="e_sb")
                    for _ in range(NT)]
            pm = [pmp.tile([128, L], F32, name="pm", tag="pm")
                  for _ in range(NT)]
            for a in range(NT):
                pe = psE.tile([128, L], F32, name="psE", tag="psE")
                k = None
                for c in range(HT):
                    inst = nc.tensor.matmul(
                        pe[:],
                        xt1t[:, L * c + 128 * a : L * c + 128 * (a + 1)],
                        xt2t[:, L * c : L * (c + 1)],
                        start=(c == 0),
                        stop=False,
                    )
                    if c == 0:
                        k = gate("psE", 2, inst)
                # m2 sentinel rank-1 (uniform -SENT on padded j columns)
                nc.tensor.matmul(pe[:], onesb[0:1, :], m2row_b,
                                 start=False, stop=True)
                # negmax over j (valid j exist; sentinel excludes padded j)
                nc.vector.reduce_max(nm4[:, a : a + 1], pe[:], axis=AX,
                                     negate=True)
                # e_sb = e + m2sent (+ m1 sentinel baked per-partition)
                nc.vector.tensor_scalar_add(
                    e_sb[a][:], pe[:],
                    m1cs[:, NT * b + a : NT * b + a + 1])
                set_gate("psE", k, touch(e_sb[a]))
                # per-chunk column max over i (m1 sentinel excludes masked i)
                nc.gpsimd.partition_all_reduce(
                    pm[a][:], e_sb[a][:], 128, bass_isa.ReduceOp.max)

            # nm4 -> row layout [1, 512] (per-column PE transposes, bf16 copy)
            nmps = psS.tile([1, L], F32, name="nmps", tag="psS")
            knm = None
            for a in range(NT):
                inst = nc.tensor.transpose(
                    nmps[0:1, 128 * a : 128 * (a + 1)], nm4[:, a : a + 1],
                    ident[:])
                if a == 0:
                    knm = gate("psS", 1, inst)
            nm1r = st.tile([1, L], BF16, name="nm1r", tag="nm1r")
            nc.vector.tensor_copy(nm1r[:], nmps[:])
            set_gate("psS", knm, touch(nm1r))
            return e_sb, pm, nm1r

        nxt = load_batch(0)
        heads = {}
        for b in range(BPC):
            xt1t, xt2t, xb1t, xb2t = nxt
            if b + 1 < BPC:
                nxt = load_batch(b + 1)

            m2row_b = m2r[0:1, L * b : L * (b + 1)]
            m1rowneg_b = m1rn[0:1, L * b : L * (b + 1)]

            if b in heads:
                e_sb, pm, nm1r = heads.pop(b)
            else:
                e_sb, pm, nm1r = emit_head(b, xt1t, xt2t)

            # ---- e^T tiles: fp32 transpose + p12T (+ z1 partial sums) ----
            p12T = [pp.tile([128, L], BF16, name="p12T", tag="p12T")
                    for _ in range(NT)]
            z1ps = psS.tile([128, NT], F32, name="z1ps", tag="psS")
            kz1 = None
            for ci, c in enumerate(reversed(range(NT))):
                tt = psT.tile([128, L], F32, name="psT", tag="psT")
                k = None
                for a in range(NT):
                    # one accumulation group for the whole bank: the first
                    # transpose starts (marks the bank pending-zero), the
                    # rest overwrite their still-pending columns
                    inst = nc.tensor.matmul(
                        tt[:, 128 * a : 128 * (a + 1)],
                        e_sb[a][:, 128 * c : 128 * (c + 1)],
                        ident[:], is_transpose=True,
                        start=(a == 0), stop=False,
                    )
                    if a == 0:
                        k = gate("psT", 2, inst)
                # undo m1 sentinel on free i, then subtract rowmax_i
                nc.tensor.matmul(tt[:], onesb[0:1, :], m1rowneg_b,
                                 start=False, stop=False)
                nc.tensor.matmul(tt[:], onesb[0:1, :], nm1r[0:1, :],
                                 start=False, stop=True)
                nc.scalar.activation(p12T[c][:], tt[:], Exp)
                set_gate("psT", k, touch(p12T[c]))
                for a in range(NT):
                    inst = nc.tensor.matmul(
                        z1ps[:, a : a + 1],
                        p12T[c][:, 128 * a : 128 * (a + 1)],
                        onescol[:], start=(ci == 0 and a == 0),
                        stop=(ci == NT - 1 and a == NT - 1))
                    if ci == 0 and a == 0:
                        kz1 = gate("psS", 1, inst)

            # combine the 4 partial column maxes, clean off the m2 sentinel
            # (keeps the value bf16-representable), negate -> ncmr row
            cm1 = st.tile([1, L], F32, name="cm1", tag="cm1")
            cm2 = st.tile([1, L], F32, name="cm2", tag="cm2")
            cm3 = st.tile([1, L], F32, name="cm3", tag="cm3")
            cm4 = st.tile([1, L], F32, name="cm4", tag="cm4")
            nc.vector.tensor_tensor(cm1[:], pm[0][0:1, :], pm[1][0:1, :],
                                    op=mybir.AluOpType.max)
            nc.vector.tensor_tensor(cm2[:], pm[2][0:1, :], pm[3][0:1, :],
                                    op=mybir.AluOpType.max)
            nc.vector.tensor_tensor(cm3[:], cm1[:], cm2[:],
                                    op=mybir.AluOpType.max)
            nc.vector.tensor_scalar_mul(cm4[:], cm3[:], -1.0)
            # broadcast the (negated, raw) column-max row to all partitions;
            # the m2 sentinel it carries cancels exactly in fp32 against the
            # same sentinel baked in e_sb
            stab = stp.tile([128, L], F32, name="stab", tag="stab")
            nc.gpsimd.partition_broadcast(stab[:], cm4[:], 128)

            # ---- p21T: restage e into psum, add stabilizer, exp (+ z2) ----
            p21T = [pp.tile([128, L], BF16, name="p21T", tag="p21T")
                    for _ in range(NT)]
            z2ps = psS.tile([128, NT], F32, name="z2ps", tag="psS")
            kz2 = None
            defer_z2 = (b == 0)
            for ai, a in enumerate(reversed(range(NT))):
                # e_sb += stab row (in place; e_sb has no later readers).
                # m1col sentinel rides along; the true -1e30 bias below
                # dominates it on masked i rows
                nc.vector.tensor_add(e_sb[a][:], e_sb[a][:], stab[:])
                nc.scalar.activation(
                    p21T[a][:], e_sb[a][:], Exp,
                    bias=m1c[:, NT * b + a : NT * b + a + 1])
                touch(p21T[a])
                if not defer_z2:
                    for c in range(NT):
                        inst = nc.tensor.matmul(
                            z2ps[:, c : c + 1],
                            p21T[a][:, 128 * c : 128 * (c + 1)],
                            onescol[:], start=(ai == 0 and c == 0),
                            stop=(ai == NT - 1 and c == NT - 1))
                        if ai == 0 and c == 0:
                            kz2 = gate("psS", 1, inst)

            if b == 0:
                # hoist batch 1's e-phase into batch 0's p21/stage-2 window
                # (batch 0 has no earlier work to hide those latency chains);
                # the z2 matmuls are deferred past it so PE isn't queued
                # behind the p21 exp chain
                with tc.high_priority(offset=2000):
                    heads[1] = emit_head(1, nxt[0], nxt[1])
                for a in range(NT):
                    for c in range(NT):
                        inst = nc.tensor.matmul(
                            z2ps[:, c : c + 1],
                            p21T[a][:, 128 * c : 128 * (c + 1)],
                            onescol[:], start=(a == 0 and c == 0),
                            stop=(a == NT - 1 and c == NT - 1))
                        if a == 0 and c == 0:
                            kz2 = gate("psS", 1, inst)

            rz1 = st.tile([128, NT], F32, name="rz1", tag="rz1")
            nc.vector.reciprocal(rz1[:], z1ps[:])
            set_gate("psS", kz1, touch(rz1))

            rz2 = st.tile([128, NT], F32, name="rz2", tag="rz2")
            nc.vector.reciprocal(rz2[:], z2ps[:])
            set_gate("psS", kz2, touch(rz2))

            # stage-2 value operands (loaded early, only now needed by PE)
            touch(xb1t)
            touch(xb2t)

            # ---- stage 2 + enhance + output ----
            for oi, (pT, xval, xnat, rz, y) in enumerate((
                (p12T, xb2t, xb1t, rz1, y1),
                (p21T, xb1t, xb2t, rz2, y2),
            )):
                for a in range(NT):
                    ys = yp.tile([128, 3 * H], BF16, name="ys", tag="ys")
                    for n in range(2):
                        # alternate between the psB and psT rings (psT is
                        # idle during stage 2) so PE can run four groups
                        # ahead of the Act normalizes
                        gid = oi * 2 * NT + 2 * a + n
                        pool, tg = (psT, "psT") if gid % 2 == 0 else (psB, "psB")
                        pt = pool.tile([128, 512], F32, name="psB", tag=tg)
                        k = None
                        for c in range(NT):
                            inst = nc.tensor.matmul(
                                pt[:],
                                pT[c][:, 128 * a : 128 * (a + 1)],
                                xval[:, H * c + 512 * n : H * c + 512 * (n + 1)],
                                start=(c == 0),
                                stop=(c == NT - 1),
                            )
                            if c == 0:
                                k = gate(tg, 2, inst)
                        nc.scalar.activation(
                            ys[:, 512 * n : 512 * (n + 1)], pt[:], Copy,
                            scale=rz[:, a : a + 1])
                        set_gate(tg, k, touch(ys[:, 512 * n : 512 * (n + 1)]))
                    xn = xnat[:, H * a : H * (a + 1)]
                    nc.vector.tensor_sub(ys[:, H : 2 * H], xn, ys[:, 0:H])
                    nc.vector.tensor_mul(ys[:, 2 * H : 3 * H], xn, ys[:, 0:H])
                    rows = slice(128 * a, 128 * (a + 1))
                    if b == BPC - 1 and oi == 1 and a == NT - 1:
                        # last tile: split across all queues to cut the tail
                        for qq in range(3):
                            dmaqs[qq].dma_start(
                                y[b, rows, qq * H : (qq + 1) * H],
                                ys[:, qq * H : (qq + 1) * H])
                    else:
                        # outputs only on SP and Pool; the Act queue must
                        # stay free for Exp/normalize
                        qi = [0, 2, 0, 2, 2, 0, 2, 0][oi * NT + a]
                        dmaqs[qi].dma_start(y[b, rows, :], ys[:])
    if not nc.is_finalized():
        nc.finalize()
    return nc


def kernel(x1_bar, seq_lengths1, x2_bar, seq_lengths2):
    x1_bar = np.ascontiguousarray(x1_bar, dtype=np.float32)
    x2_bar = np.ascontiguousarray(x2_bar, dtype=np.float32)
    sl1 = np.asarray(seq_lengths1).astype(np.int32)
    sl2 = np.asarray(seq_lengths2).astype(np.int32)

    xb1 = x1_bar.astype(NPBF16)
    xb2 = x2_bar.astype(NPBF16)
    xt1 = np.ascontiguousarray(x1_bar.transpose(0, 2, 1))
    xt2 = np.ascontiguousarray(x2_bar.transpose(0, 2, 1))

    ar = np.arange(L, dtype=np.int32)
    pad1 = ar[None, :] >= sl1[:, None]  # [B, L] True on padded i
    pad2 = ar[None, :] >= sl2[:, None]
    m2row = np.where(pad2, -SENT, 0.0).astype(NPBF16)
    m1rowneg = np.where(pad1, SENT, 0.0).astype(NPBF16)
    # col masks, swizzled to [128, B*NT]: col[p, b*NT+a] = mask[b, a*128+p]
    def swz(m, val):
        out = np.where(m, val, 0.0).astype(np.float32)  # [B, L]
        return np.ascontiguousarray(
            out.reshape(B, NT, 128).transpose(2, 0, 1).reshape(128, B * NT))
    m1col = swz(pad1, NEG)
    m1colsent = swz(pad1, -SENT)

    if "nc" not in _NC_CACHE:
        _NC_CACHE["nc"] = build_nc()
    nc = _NC_CACHE["nc"]

    in_maps = []
    for c in range(NCORES):
        s = slice(c * BPC, (c + 1) * BPC)
        sc = slice(c * BPC * NT, (c + 1) * BPC * NT)
        in_maps.append({
            "xb1": xb1[s], "xb2": xb2[s], "xt1": xt1[s], "xt2": xt2[s],
            "m2row": m2row[s], "m1rowneg": m1rowneg[s],
            "m1col": m1col[:, sc], "m1colsent": m1colsent[:, sc],
        })

    res = run_bass_kernel_spmd(nc, in_maps, core_ids=list(range(NCORES)))
    yd1 = np.concatenate([r["y1"] for r in res.results], axis=0)
    yd2 = np.concatenate([r["y2"] for r in res.results], axis=0)

    y1 = np.empty((B, L, 4 * H), dtype=np.float32)
    y2 = np.empty((B, L, 4 * H), dtype=np.float32)
    y1[:, :, 0:H] = x1_bar
    y2[:, :, 0:H] = x2_bar
    y1[:, :, H:] = yd1.astype(np.float32)
    y2[:, :, H:] = yd2.astype(np.float32)
    return y1, y2



# revision 15
# speedup vs baseline: 1.1382x; 1.1382x over previous
"""Trainium2 Bass kernel for nn_LocalInferenceModeling (cross-attention enhance).

Reference computation (per batch b):
    e = x1 @ x2^T                                  [L, L]
    a12 = softmax_j(e + m2[j]);  x1t = a12 @ x2    [L, H]
    a21 = softmax_i(e^T + m1[i]); x2t = a21 @ x1   [L, H]
    y1 = concat([x1, x1t, x1 - x1t, x1 * x1t], -1) [L, 4H]
    y2 = concat([x2, x2t, x2 - x2t, x2 * x2t], -1)

Sharding: batch dim B=32 split across 8 NeuronCores (4 batches/core), no
communication.  The device computes x1_tilde / x2_tilde; the host performs
the final (elementwise) enhance/concat on the exact fp32 inputs.

Device-side design:
  - e is computed once, in fp32 (f32r matmuls at full PE rate), in natural
    [i, j] layout.  A fused DVE tensor_tensor_reduce adds the pad masks
    (bf16-exact sentinel -29952 on padded j columns via a gpsimd partition
    broadcast of the mask row, plus per-partition sentinel on padded i) and
    emits the per-row max in the same pass.
  - p12 = exp(e - rowmax) runs on the Activation engine with a per-partition
    bias, emitting the softmax denominator z1 via accum_out for free.  The
    probabilities are normalized in fp16 (x0.5 DVE cost), transposed on the
    PE at 1 cycle/row (fp16), and contracted against fp16 x2 values.  The
    resulting psum is final (already normalized) and is DMA'd straight from
    PSUM to HBM in fp32.
  - p21 reuses e: column max via gpsimd partition reduces over valid-i tiles
    (the j-sentinel rides along and cancels exactly in fp32 when subtracted),
    exp with a true -1e30 per-partition bias for padded i, z2 via rank-1 PE
    matmuls, normalization folded into the psum->SBUF copies (spread across
    Pool/DVE/Act), fp16 output.
  - Sequence-length sparsity: softmax probabilities of fully-padded 128-row
    chunks are exactly zero, so the stage-2 contractions only run over the
    first C1/C2 chunks.  All 8 cores share one program, so the per-slot
    chunk counts are baked as the max over the cores' batches after a
    host-side assignment that groups batches of similar length; the program
    is rebuilt (and cached) per distinct slot signature.
"""

import sys

import numpy as np

sys.path.insert(0, "/opt/trn_rl_repo")

from contextlib import ExitStack

import concourse.bass as bass
import concourse.bacc as bacc
import concourse.bass_isa as bass_isa
import concourse.mybir as mybir
from concourse import masks
from concourse.bass_utils import run_bass_kernel_spmd
from concourse.tile import TileContext

B, L, H = 32, 512, 1024
NCORES = 8
BPC = B // NCORES  # batches per core
NT = L // 128  # 4 partition tiles per L
HT = H // 128  # 8 partition tiles per H

SENT = np.float32(29952.0)  # bf16-exact sentinel magnitude
NEG = np.float32(-1.0e30)

F32 = mybir.dt.float32
F32R = mybir.dt.float32r
FP16 = mybir.dt.float16

Exp = mybir.ActivationFunctionType.Exp
Copy = mybir.ActivationFunctionType.Copy
Add = mybir.AluOpType.add
Max = mybir.AluOpType.max

_NC_CACHE = {}


def build_nc(slots):
    """slots: tuple of BPC (C1, C2) pairs; C = valid 128-chunk count baked
    into slot k of every core."""
    nc = bacc.Bacc(None, target_bir_lowering=False)
    xt1 = nc.dram_tensor("xt1", [BPC, H, L], F32R, kind="ExternalInput")
    xt2 = nc.dram_tensor("xt2", [BPC, H, L], F32R, kind="ExternalInput")
    xb1 = nc.dram_tensor("xb1", [BPC, L, H], FP16, kind="ExternalInput")
    xb2 = nc.dram_tensor("xb2", [BPC, L, H], FP16, kind="ExternalInput")
    m2row = nc.dram_tensor("m2row", [BPC, L], F32, kind="ExternalInput")
    m1cs = nc.dram_tensor("m1cs", [128, BPC * NT], F32, kind="ExternalInput")
    m1c = nc.dram_tensor("m1c", [128, BPC * NT], F32, kind="ExternalInput")
    o1 = nc.dram_tensor("o1", [BPC, L, H], FP16, kind="ExternalOutput")
    o2 = nc.dram_tensor("o2", [BPC, L, H], FP16, kind="ExternalOutput")

    qSP, qACT, qPL = nc.sync, nc.scalar, nc.gpsimd

    with TileContext(nc) as tc, ExitStack() as ctx:
        from concourse.tile import add_dep_helper

        const = ctx.enter_context(tc.tile_pool(name="const", bufs=1))
        ident = const.tile([128, 128], FP16)
        masks.make_identity(nc, ident[:])
        onesh = const.tile([128, 1], FP16)
        nc.vector.memset(onesh[:], 1.0)
        ones32col = const.tile([128, 1], F32)
        nc.vector.memset(ones32col[:], 1.0)
        ones32 = const.tile([1, 32], F32)
        nc.vector.memset(ones32[:], 1.0)

        xp = ctx.enter_context(tc.tile_pool(name="xp", bufs=2))
        esb = ctx.enter_context(tc.tile_pool(name="esb", bufs=6))
        mbp = ctx.enter_context(tc.tile_pool(name="mbp", bufs=2))
        map_ = ctx.enter_context(tc.tile_pool(name="map", bufs=6))
        pmp = ctx.enter_context(tc.tile_pool(name="pmp", bufs=6))
        cmp_ = ctx.enter_context(tc.tile_pool(name="cmp", bufs=2))
        pp = ctx.enter_context(tc.tile_pool(name="pp", bufs=2 * NT))
        ptp = ctx.enter_context(tc.tile_pool(name="ptp", bufs=2 * NT))
        p21p = ctx.enter_context(tc.tile_pool(name="p21p", bufs=2 * NT))
        st = ctx.enter_context(tc.tile_pool(name="st", bufs=6))
        o2p = ctx.enter_context(tc.tile_pool(name="o2p", bufs=6))
        mrp = ctx.enter_context(tc.tile_pool(name="mrp", bufs=1))
        psE = ctx.enter_context(tc.tile_pool(name="psE", bufs=2, space="PSUM"))
        psT = ctx.enter_context(tc.tile_pool(name="psT", bufs=2, space="PSUM"))
        psS = ctx.enter_context(tc.tile_pool(name="psS", bufs=3, space="PSUM"))
        psScr = ctx.enter_context(
            tc.tile_pool(name="psScr", bufs=1, space="PSUM"))
        scratch = psScr.tile([32, 32], F32, name="scratch", tag="scratch")

        gates = {"psE": [], "psT": [], "psS": []}
        touch_cnt = [0]

        def touch(ap):
            # Tiny PE matmul reading `ap` so the PE engine observes the
            # producer's sem tick; real matmuls then carry at most one sync
            # wait. Rotate over scratch columns so touches don't WAW-chain.
            p = min(ap.shape[0], 32)
            f = min(ap.shape[1], 32)
            if ap.dtype == F32R:
                ap = ap.bitcast(F32)
            oc = onesh if ap.dtype == FP16 else ones32col
            col = touch_cnt[0] % 32
            touch_cnt[0] += 1
            with tc.high_priority(offset=200):
                return nc.tensor.matmul(
                    scratch[0:f, col : col + 1], ap[0:p, 0:f], oc[0:p, 0:1],
                    start=True, stop=True)

        def gate(tag, bufs, first_inst):
            # Order the group's first PE write after the touch that observed
            # the release of the slot it reuses (bufs groups back).
            hist = gates[tag]
            k = len(hist)
            if k >= bufs and hist[k - bufs] is not None:
                add_dep_helper(first_inst.ins, hist[k - bufs].ins, sync=False,
                               reason="psum slot gate")
            hist.append(None)
            return k

        def set_gate(tag, k, tinst):
            gates[tag][k] = tinst

        touch(ident)
        nc.tensor.matmul(scratch[0:32, 0:1], ones32[0:1, :], ones32[0:1, 0:1],
                         start=True, stop=True)

        # ---- static mask loads ----
        m2r = mrp.tile([1, BPC * L], F32, name="m2r", tag="m2r")
        m1cst = mrp.tile([128, BPC * NT], F32, name="m1cst", tag="m1cst")
        m1ct = mrp.tile([128, BPC * NT], F32, name="m1ct", tag="m1ct")
        qACT.dma_start(m2r[:1, :], m2row.rearrange("b l -> (b l)")[None, :])
        qACT.dma_start(m1cst[:], m1cs[:, :])
        qACT.dma_start(m1ct[:], m1c[:, :])

        def load_batch(b):
            C1, C2 = slots[b]
            xt1t = xp.tile([128, HT * L], F32R, name="xt1t", tag="xt1t")
            xt2t = xp.tile([128, HT * L], F32R, name="xt2t", tag="xt2t")
            xb1t = xp.tile([128, NT * H], FP16, name="xb1t", tag="xb1t")
            xb2t = xp.tile([128, NT * H], FP16, name="xb2t", tag="xb2t")
            # transposed e operands, split in half so the first e matmuls
            # can start at half-load
            hh = HT // 2
            for q, t, src in ((qSP, xt1t, xt1), (qPL, xt2t, xt2)):
                q.dma_start(
                    t[:, : hh * L].rearrange("p (c l) -> p c l", c=hh),
                    src[b, : hh * 128].rearrange("(c p) l -> p c l", p=128))
                q.dma_start(
                    t[:, hh * L :].rearrange("p (c l) -> p c l", c=hh),
                    src[b, hh * 128 :].rearrange("(c p) l -> p c l", p=128))
            # natural stage-2 values: only the valid chunks are ever read
            qACT.dma_start(
                xb1t[:, : C1 * H].rearrange("p (a h) -> p a h", a=C1),
                xb1[b, : C1 * 128].rearrange("(a p) h -> p a h", p=128))
            qACT.dma_start(
                xb2t[:, : C2 * H].rearrange("p (a h) -> p a h", a=C2),
                xb2[b, : C2 * 128].rearrange("(a p) h -> p a h", p=128))
            return xt1t, xt2t, xb1t, xb2t

        def emit_head(b, xt1t, xt2t):
            """e matmuls + masks/rowmax + p12 (normalized fp16) + colmax cm
            + p21 + z2.  Returns what stage 2 needs."""
            C1, C2 = slots[b]
            touch(xt1t)
            touch(xt2t)

            m2b = mbp.tile([128, L], F32, name="m2b", tag="m2b")
            nc.gpsimd.partition_broadcast(
                m2b[:], m2r[0:1, L * b : L * (b + 1)], 128)
            mba = []
            for a in range(C1):
                t = map_.tile([128, L], F32, name="mba", tag="mba")
                nc.gpsimd.tensor_scalar_add(
                    t[:], m2b[:], m1cst[:, NT * b + a : NT * b + a + 1])
                mba.append(t)

            nm4 = st.tile([128, 2 * NT], F32, name="nm4", tag="nm4")
            z1 = st.tile([128, 2 * NT], F32, name="z1", tag="z1")
            e_sb = [esb.tile([128, L], F32, name="e_sb", tag="e_sb")
                    for _ in range(NT)]
            p12 = [pp.tile([128, L], FP16, name="p12", tag="p12")
                   for _ in range(NT)]
            pm = [pmp.tile([128, L], F32, name="pm", tag="pm")
                  for _ in range(C1)]
            W2 = C2 * 128
            for a in range(NT):
                pe = psE.tile([128, L], F32, name="psE", tag="psE")
                k = None
                for c in range(HT):
                    inst = nc.tensor.matmul(
                        pe[:],
                        xt1t[:, L * c + 128 * a : L * c + 128 * (a + 1)],
                        xt2t[:, L * c : L * (c + 1)],
                        start=(c == 0),
                        stop=(c == HT - 1),
                    )
                    if c == 0:
                        k = gate("psE", 2, inst)
                mb = mba[a] if a < C1 else m2b
                # e_sb = e + masks; rowmax emitted in the same DVE pass
                nc.vector.tensor_tensor_reduce(
                    out=e_sb[a][:], in0=pe[:], in1=mb[:], scale=1.0,
                    scalar=-3.0e38, op0=Add, op1=Max,
                    accum_out=nm4[:, a : a + 1])
                set_gate("psE", k, touch(e_sb[a]))
                nc.vector.tensor_scalar_mul(
                    nm4[:, NT + a : NT + a + 1], nm4[:, a : a + 1], -1.0)
                # p12 = exp(e_sb - rowmax) over valid-j chunks; z1 for free
                nc.scalar.activation(
                    p12[a][:, :W2], e_sb[a][:, :W2], Exp,
                    bias=nm4[:, NT + a : NT + a + 1],
                    accum_out=z1[:, a : a + 1])
                # rz1 lands in the upper half of z1
                nc.vector.reciprocal(z1[:, NT + a : NT + a + 1],
                                     z1[:, a : a + 1])
                touch(p12[a])
                if a < C1:
                    nc.gpsimd.partition_all_reduce(
                        pm[a][:], e_sb[a][:], 128, bass_isa.ReduceOp.max)

            # column max over valid i (sentinels cancel on subtraction)
            if C1 == 1:
                cm = pm[0]
            else:
                cm = cmp_.tile([128, L], F32, name="cm", tag="cm")
                nc.vector.tensor_max(cm[:], pm[0][:], pm[1][:])
                for a in range(2, C1):
                    nc.vector.tensor_max(cm[:], cm[:], pm[a][:])

            # p21 = exp(e - colmax) with -1e30 bias on padded i; z2 via PE
            p21 = [p21p.tile([128, L], FP16, name="p21", tag="p21")
                   for _ in range(C1)]
            z2ps = psS.tile([128, NT], F32, name="z2ps", tag="psS")
            kz2 = None
            for a in range(C1):
                nc.vector.tensor_sub(e_sb[a][:], e_sb[a][:], cm[:])
                nc.scalar.activation(
                    p21[a][:], e_sb[a][:], Exp,
                    bias=m1ct[:, NT * b + a : NT * b + a + 1])
                touch(p21[a])
                for t in range(NT):
                    inst = nc.tensor.matmul(
                        z2ps[:, t : t + 1],
                        p21[a][:, 128 * t : 128 * (t + 1)],
                        onesh[:], start=(a == 0 and t == 0),
                        stop=(a == C1 - 1 and t == NT - 1))
                    if a == 0 and t == 0:
                        kz2 = gate("psS", 3, inst)
            rz2 = st.tile([128, NT], F32, name="rz2", tag="rz2")
            nc.vector.reciprocal(rz2[:], z2ps[:])
            set_gate("psS", kz2, touch(rz2))
            return p12, p21, z1, rz2

        def emit_stage2(b, head, xb1t, xb2t):
            C1, C2 = slots[b]
            p12, p21, z1, rz2 = head
            touch(xb1t)
            touch(xb2t)

            # p12T[c] = transpose of normalized p12 chunks (fp16, exact)
            p12T = []
            for c in range(C2):
                tp = psT.tile([128, L], FP16, name="psT", tag="psT")
                k = None
                for a in range(NT):
                    inst = nc.tensor.matmul(
                        tp[:, 128 * a : 128 * (a + 1)],
                        p12[a][:, 128 * c : 128 * (c + 1)],
                        ident[:], is_transpose=True,
                        start=(a == 0), stop=False,
                    )
                    if a == 0:
                        k = gate("psT", 2, inst)
                sb = ptp.tile([128, L], FP16, name="p12T", tag="p12T")
                nc.gpsimd.tensor_copy(sb[:], tp[:])
                set_gate("psT", k, touch(sb))
                p12T.append(sb)

            # interleave x1t and x2t output tiles; normalization (per
            # out-partition 1/z) folds into the psum->SBUF copies, spread
            # round-robin over Pool/DVE/Act
            cp_engs = [nc.gpsimd, nc.vector, nc.scalar]
            cp_i = [0]

            def norm_copy(dst, pt, rz):
                eng = cp_engs[cp_i[0] % 3]
                cp_i[0] += 1
                if eng is nc.scalar:
                    eng.activation(dst, pt, Copy, scale=rz)
                else:
                    eng.tensor_scalar_mul(dst, pt, rz)

            for a in range(NT):
                ys1 = o2p.tile([128, H], FP16, name="ys1", tag="ys1")
                for n in range(2):
                    pt = psS.tile([128, 512], F32, name="psS", tag="psS")
                    k = None
                    for ci in range(C2):
                        inst = nc.tensor.matmul(
                            pt[:],
                            p12T[ci][:, 128 * a : 128 * (a + 1)],
                            xb2t[:, H * ci + 512 * n : H * ci + 512 * (n + 1)],
                            start=(ci == 0), stop=(ci == C2 - 1),
                        )
                        if ci == 0:
                            k = gate("psS", 3, inst)
                    norm_copy(ys1[:, 512 * n : 512 * (n + 1)], pt[:],
                              z1[:, NT + a : NT + a + 1])
                    set_gate("psS", k, touch(ys1[:, 512 * n : 512 * (n + 1)]))
                qSP.dma_start(o1[b, 128 * a : 128 * (a + 1), :], ys1[:])
                # x2t output tile t == a
                ys = o2p.tile([128, H], FP16, name="ys", tag="ys")
                for n in range(2):
                    pt = psS.tile([128, 512], F32, name="psS", tag="psS")
                    k = None
                    for ai in range(C1):
                        inst = nc.tensor.matmul(
                            pt[:],
                            p21[ai][:, 128 * a : 128 * (a + 1)],
                            xb1t[:, H * ai + 512 * n : H * ai + 512 * (n + 1)],
                            start=(ai == 0), stop=(ai == C1 - 1),
                        )
                        if ai == 0:
                            k = gate("psS", 3, inst)
                    norm_copy(ys[:, 512 * n : 512 * (n + 1)], pt[:],
                              rz2[:, a : a + 1])
                    set_gate("psS", k, touch(ys[:, 512 * n : 512 * (n + 1)]))
                qPL.dma_start(o2[b, 128 * a : 128 * (a + 1), :], ys[:])

        # ---- software-pipelined batch loop ----
        # PE order: e(b) | stage2(b-1) | transposes+z2(b) | e(b+1) | ...
        nxt = load_batch(0)
        heads = {}
        xbs = {0: (nxt[2], nxt[3])}
        for b in range(BPC):
            xt1t, xt2t, xb1t, xb2t = nxt
            xbs[b] = (xb1t, xb2t)
            heads[b] = emit_head(b, xt1t, xt2t)
            if b + 1 < BPC:
                nxt = load_batch(b + 1)
            if b > 0:
                emit_stage2(b - 1, heads.pop(b - 1), *xbs.pop(b - 1))
        emit_stage2(BPC - 1, heads.pop(BPC - 1), *xbs.pop(BPC - 1))

    if not nc.is_finalized():
        nc.finalize()
    return nc


def _plan_slots(c1, c2):
    """Partition the B batches into BPC groups of NCORES, minimizing
    sum over groups of (max c1 + max c2).  Returns (slots, assign) where
    assign[core][slot] = original batch index."""
    order = np.argsort(-(c1 + c2), kind="stable")
    groups = [list(order[k * NCORES : (k + 1) * NCORES]) for k in range(BPC)]

    def gcost(g):
        return max(c1[i] for i in g) + max(c2[i] for i in g)

    # local search: swap members between groups while it helps
    improved = True
    it = 0
    while improved and it < 200:
        improved = False
        it += 1
        for ga in range(BPC):
            for gb in range(ga + 1, BPC):
                base = gcost(groups[ga]) + gcost(groups[gb])
                for ia in range(NCORES):
                    for ib in range(NCORES):
                        groups[ga][ia], groups[gb][ib] = (
                            groups[gb][ib], groups[ga][ia])
                        new = gcost(groups[ga]) + gcost(groups[gb])
                        if new < base:
                            base = new
                            improved = True
                        else:
                            groups[ga][ia], groups[gb][ib] = (
                                groups[gb][ib], groups[ga][ia])
    slots = tuple(
        (int(max(c1[i] for i in g)), int(max(c2[i] for i in g)))
        for g in groups)
    assign = [[groups[k][core] for k in range(BPC)]
              for core in range(NCORES)]
    return slots, assign


def kernel(x1_bar, seq_lengths1, x2_bar, seq_lengths2):
    x1_bar = np.ascontiguousarray(x1_bar, dtype=np.float32)
    x2_bar = np.ascontiguousarray(x2_bar, dtype=np.float32)
    sl1 = np.asarray(seq_lengths1).astype(np.int32)
    sl2 = np.asarray(seq_lengths2).astype(np.int32)

    c1 = np.clip((sl1 + 127) // 128, 1, NT).astype(np.int64)
    c2 = np.clip((sl2 + 127) // 128, 1, NT).astype(np.int64)
    slots, assign = _plan_slots(c1, c2)

    xt1f = np.ascontiguousarray(x1_bar.transpose(0, 2, 1))
    xt2f = np.ascontiguousarray(x2_bar.transpose(0, 2, 1))
    xb1f = x1_bar.astype(np.float16)
    xb2f = x2_bar.astype(np.float16)

    ar = np.arange(L, dtype=np.int32)
    pad1 = ar[None, :] >= sl1[:, None]  # [B, L] True on padded i
    pad2 = ar[None, :] >= sl2[:, None]
    m2rowf = np.where(pad2, -SENT, 0.0).astype(np.float32)

    def swz(m, val, idx):
        out = np.where(m[idx], val, 0.0).astype(np.float32)  # [BPC, L]
        return np.ascontiguousarray(
            out.reshape(BPC, NT, 128).transpose(2, 0, 1).reshape(
                128, BPC * NT))

    key = slots
    if key not in _NC_CACHE:
        _NC_CACHE.clear()
        _NC_CACHE[key] = build_nc(slots)
    nc = _NC_CACHE[key]

    in_maps = []
    for core in range(NCORES):
        idx = np.array(assign[core], dtype=np.int64)
        in_maps.append({
            "xt1": np.ascontiguousarray(xt1f[idx]),
            "xt2": np.ascontiguousarray(xt2f[idx]),
            "xb1": np.ascontiguousarray(xb1f[idx]),
            "xb2": np.ascontiguousarray(xb2f[idx]),
            "m2row": np.ascontiguousarray(m2rowf[idx]),
            "m1cs": swz(pad1, -SENT, idx),
            "m1c": swz(pad1, NEG, idx),
        })

    res = run_bass_kernel_spmd(nc, in_maps, core_ids=list(range(NCORES)))

    x1t = np.empty((B, L, H), dtype=np.float32)
    x2t = np.empty((B, L, H), dtype=np.float32)
    for core in range(NCORES):
        r = res.results[core]
        for k in range(BPC):
            bi = assign[core][k]
            x1t[bi] = r["o1"][k].astype(np.float32)
            x2t[bi] = r["o2"][k].astype(np.float32)

    y1 = np.empty((B, L, 4 * H), dtype=np.float32)
    y2 = np.empty((B, L, 4 * H), dtype=np.float32)
    y1[:, :, 0:H] = x1_bar
    y1[:, :, H : 2 * H] = x1t
    y1[:, :, 2 * H : 3 * H] = x1_bar - x1t
    y1[:, :, 3 * H :] = x1_bar * x1t
    y2[:, :, 0:H] = x2_bar
    y2[:, :, H : 2 * H] = x2t
    y2[:, :, 2 * H : 3 * H] = x2_bar - x2t
    y2[:, :, 3 * H :] = x2_bar * x2t
    return y1, y2


# revision 20
# speedup vs baseline: 1.1678x; 1.0260x over previous
"""Trainium2 Bass kernel for nn_LocalInferenceModeling (cross-attention enhance).

Reference computation (per batch b):
    e = x1 @ x2^T                                  [L, L]
    a12 = softmax_j(e + m2[j]);  x1t = a12 @ x2    [L, H]
    a21 = softmax_i(e^T + m1[i]); x2t = a21 @ x1   [L, H]
    y1 = concat([x1, x1t, x1 - x1t, x1 * x1t], -1) [L, 4H]
    y2 = concat([x2, x2t, x2 - x2t, x2 * x2t], -1)

Sharding: batch dim B=32 split across 8 NeuronCores (4 batches/core), no
communication.  The device computes x1_tilde / x2_tilde; the host performs
the final (elementwise) enhance/concat on the exact fp32 inputs.

Device-side design:
  - e is computed once, in fp32 (f32r matmuls at full PE rate), in natural
    [i, j] layout.  A fused DVE tensor_tensor_reduce adds the pad masks
    (bf16-exact sentinel -29952 on padded j columns via a gpsimd partition
    broadcast of the mask row, plus per-partition sentinel on padded i) and
    emits the per-row max in the same pass.
  - p12 = exp(e - rowmax) runs on the Activation engine with a per-partition
    bias, emitting the softmax denominator z1 via accum_out for free.  The
    probabilities are normalized in fp16 (x0.5 DVE cost), transposed on the
    PE at 1 cycle/row (fp16), and contracted against fp16 x2 values.  The
    resulting psum is final (already normalized) and is DMA'd straight from
    PSUM to HBM in fp32.
  - p21 reuses e: column max via gpsimd partition reduces over valid-i tiles
    (the j-sentinel rides along and cancels exactly in fp32 when subtracted),
    exp with a true -1e30 per-partition bias for padded i, z2 via rank-1 PE
    matmuls, normalization folded into the psum->SBUF copies (spread across
    Pool/DVE/Act), fp16 output.
  - Sequence-length sparsity: softmax probabilities of fully-padded 128-row
    chunks are exactly zero, so the stage-2 contractions only run over the
    first C1/C2 chunks.  All 8 cores share one program, so the per-slot
    chunk counts are baked as the max over the cores' batches after a
    host-side assignment that groups batches of similar length; the program
    is rebuilt (and cached) per distinct slot signature.
"""

import sys

import numpy as np

sys.path.insert(0, "/opt/trn_rl_repo")

from contextlib import ExitStack

import concourse.bass as bass
import concourse.bacc as bacc
import concourse.bass_isa as bass_isa
import concourse.mybir as mybir
from concourse import masks
from concourse.bass_utils import run_bass_kernel_spmd
from concourse.tile import TileContext

B, L, H = 32, 512, 1024
NCORES = 8
BPC = B // NCORES  # batches per core
NT = L // 128  # 4 partition tiles per L
HT = H // 128  # 8 partition tiles per H

SENT = np.float32(29952.0)  # bf16-exact sentinel magnitude
NEG = np.float32(-1.0e30)

F32 = mybir.dt.float32
F32R = mybir.dt.float32r
FP16 = mybir.dt.float16

Exp = mybir.ActivationFunctionType.Exp
Copy = mybir.ActivationFunctionType.Copy
Add = mybir.AluOpType.add
Max = mybir.AluOpType.max

_NC_CACHE = {}


def build_nc(slots):
    """slots: tuple of BPC (C1, C2) pairs; C = valid 128-chunk count baked
    into slot k of every core."""
    nc = bacc.Bacc(None, target_bir_lowering=False)
    xt1 = nc.dram_tensor("xt1", [BPC, H, L], F32R, kind="ExternalInput")
    xt2 = nc.dram_tensor("xt2", [BPC, H, L], F32R, kind="ExternalInput")
    xb1 = nc.dram_tensor("xb1", [BPC, L, H], FP16, kind="ExternalInput")
    xb2 = nc.dram_tensor("xb2", [BPC, L, H], FP16, kind="ExternalInput")
    m2row = nc.dram_tensor("m2row", [BPC, L], F32, kind="ExternalInput")
    m1cs = nc.dram_tensor("m1cs", [128, BPC * NT], F32, kind="ExternalInput")
    m1c = nc.dram_tensor("m1c", [128, BPC * NT], F32, kind="ExternalInput")
    o1 = nc.dram_tensor("o1", [BPC, L, H], FP16, kind="ExternalOutput")
    o2 = nc.dram_tensor("o2", [BPC, L, H], FP16, kind="ExternalOutput")

    qSP, qACT, qPL = nc.sync, nc.scalar, nc.gpsimd

    with TileContext(nc) as tc, ExitStack() as ctx:
        from concourse.tile import add_dep_helper

        const = ctx.enter_context(tc.tile_pool(name="const", bufs=1))
        ident = const.tile([128, 128], FP16)
        masks.make_identity(nc, ident[:])
        onesh = const.tile([128, 1], FP16)
        nc.vector.memset(onesh[:], 1.0)
        ones32col = const.tile([128, 1], F32)
        nc.vector.memset(ones32col[:], 1.0)
        ones32 = const.tile([1, 32], F32)
        nc.vector.memset(ones32[:], 1.0)

        xp = ctx.enter_context(tc.tile_pool(name="xp", bufs=2))
        esb = ctx.enter_context(tc.tile_pool(name="esb", bufs=6))
        mbp = ctx.enter_context(tc.tile_pool(name="mbp", bufs=2))
        map_ = ctx.enter_context(tc.tile_pool(name="map", bufs=6))
        pmp = ctx.enter_context(tc.tile_pool(name="pmp", bufs=6))
        cmp_ = ctx.enter_context(tc.tile_pool(name="cmp", bufs=2))
        pp = ctx.enter_context(tc.tile_pool(name="pp", bufs=2 * NT))
        ptp = ctx.enter_context(tc.tile_pool(name="ptp", bufs=2 * NT))
        p21p = ctx.enter_context(tc.tile_pool(name="p21p", bufs=2 * NT))
        st = ctx.enter_context(tc.tile_pool(name="st", bufs=6))
        o2p = ctx.enter_context(tc.tile_pool(name="o2p", bufs=6))
        mrp = ctx.enter_context(tc.tile_pool(name="mrp", bufs=1))
        psE = ctx.enter_context(tc.tile_pool(name="psE", bufs=2, space="PSUM"))
        psT = ctx.enter_context(tc.tile_pool(name="psT", bufs=2, space="PSUM"))
        psS = ctx.enter_context(tc.tile_pool(name="psS", bufs=3, space="PSUM"))
        psScr = ctx.enter_context(
            tc.tile_pool(name="psScr", bufs=1, space="PSUM"))
        scratch = psScr.tile([32, 32], F32, name="scratch", tag="scratch")

        gates = {"psE": [], "psT": [], "psS": []}
        touch_cnt = [0]

        def touch(ap):
            # Tiny PE matmul reading `ap` so the PE engine observes the
            # producer's sem tick; real matmuls then carry at most one sync
            # wait. Rotate over scratch columns so touches don't WAW-chain.
            p = min(ap.shape[0], 32)
            f = min(ap.shape[1], 32)
            if ap.dtype == F32R:
                ap = ap.bitcast(F32)
            oc = onesh if ap.dtype == FP16 else ones32col
            col = touch_cnt[0] % 32
            touch_cnt[0] += 1
            with tc.high_priority(offset=200):
                return nc.tensor.matmul(
                    scratch[0:f, col : col + 1], ap[0:p, 0:f], oc[0:p, 0:1],
                    start=True, stop=True)

        def gate(tag, bufs, first_inst):
            # Order the group's first PE write after the touch that observed
            # the release of the slot it reuses (bufs groups back).
            hist = gates[tag]
            k = len(hist)
            if k >= bufs and hist[k - bufs] is not None:
                add_dep_helper(first_inst.ins, hist[k - bufs].ins, sync=False,
                               reason="psum slot gate")
            hist.append(None)
            return k

        def set_gate(tag, k, tinst):
            gates[tag][k] = tinst

        touch(ident)
        nc.tensor.matmul(scratch[0:32, 0:1], ones32[0:1, :], ones32[0:1, 0:1],
                         start=True, stop=True)

        # ---- static mask loads ----
        m2r = mrp.tile([1, BPC * L], F32, name="m2r", tag="m2r")
        m1cst = mrp.tile([128, BPC * NT], F32, name="m1cst", tag="m1cst")
        m1ct = mrp.tile([128, BPC * NT], F32, name="m1ct", tag="m1ct")
        qACT.dma_start(m2r[:1, :], m2row.rearrange("b l -> (b l)")[None, :])
        qACT.dma_start(m1cst[:], m1cs[:, :])
        qACT.dma_start(m1ct[:], m1c[:, :])

        def load_batch(b):
            C1, C2 = slots[b]
            xt1t = xp.tile([128, HT * L], F32R, name="xt1t", tag="xt1t")
            xt2t = xp.tile([128, HT * L], F32R, name="xt2t", tag="xt2t")
            xb1t = xp.tile([128, NT * H], FP16, name="xb1t", tag="xb1t")
            xb2t = xp.tile([128, NT * H], FP16, name="xb2t", tag="xb2t")
            # transposed e operands, split in half so the first e matmuls
            # can start at half-load.  batch 1's halves borrow the Act queue
            # so the pipeline prologue (two batches of loads, no outputs yet)
            # spreads over three queues instead of two.
            hh = HT // 2
            if b == 1:
                q_assign = ((qACT, qSP, xt1t, xt1), (qPL, qACT, xt2t, xt2))
            else:
                q_assign = ((qSP, qSP, xt1t, xt1), (qPL, qPL, xt2t, xt2))
            for qa, qb_, t, src in q_assign:
                qa.dma_start(
                    t[:, : hh * L].rearrange("p (c l) -> p c l", c=hh),
                    src[b, : hh * 128].rearrange("(c p) l -> p c l", p=128))
                qb_.dma_start(
                    t[:, hh * L :].rearrange("p (c l) -> p c l", c=hh),
                    src[b, hh * 128 :].rearrange("(c p) l -> p c l", p=128))
            # natural stage-2 values: only the valid chunks are ever read
            qACT.dma_start(
                xb1t[:, : C1 * H].rearrange("p (a h) -> p a h", a=C1),
                xb1[b, : C1 * 128].rearrange("(a p) h -> p a h", p=128))
            qACT.dma_start(
                xb2t[:, : C2 * H].rearrange("p (a h) -> p a h", a=C2),
                xb2[b, : C2 * 128].rearrange("(a p) h -> p a h", p=128))
            return xt1t, xt2t, xb1t, xb2t

        def emit_head(b, xt1t, xt2t):
            """e matmuls + masks/rowmax + p12 (normalized fp16) + colmax cm
            + p21 + z2.  Returns what stage 2 needs."""
            C1, C2 = slots[b]
            touch(xt1t)
            touch(xt2t)

            m2b = mbp.tile([128, L], F32, name="m2b", tag="m2b")
            nc.gpsimd.partition_broadcast(
                m2b[:], m2r[0:1, L * b : L * (b + 1)], 128)
            mba = []
            for a in range(C1):
                t = map_.tile([128, L], F32, name="mba", tag="mba")
                nc.gpsimd.tensor_scalar_add(
                    t[:], m2b[:], m1cst[:, NT * b + a : NT * b + a + 1])
                mba.append(t)

            nm4 = st.tile([128, 2 * NT], F32, name="nm4", tag="nm4")
            z1 = st.tile([128, 2 * NT], F32, name="z1", tag="z1")
            e_sb = [esb.tile([128, L], F32, name="e_sb", tag="e_sb")
                    for _ in range(NT)]
            p12 = [pp.tile([128, L], FP16, name="p12", tag="p12")
                   for _ in range(NT)]
            pm = [pmp.tile([128, L], F32, name="pm", tag="pm")
                  for _ in range(C1)]
            W2 = C2 * 128
            for a in range(NT):
                pe = psE.tile([128, L], F32, name="psE", tag="psE")
                k = None
                for c in range(HT):
                    inst = nc.tensor.matmul(
                        pe[:],
                        xt1t[:, L * c + 128 * a : L * c + 128 * (a + 1)],
                        xt2t[:, L * c : L * (c + 1)],
                        start=(c == 0),
                        stop=(c == HT - 1),
                    )
                    if c == 0:
                        k = gate("psE", 2, inst)
                mb = mba[a] if a < C1 else m2b
                # e_sb = e + masks; rowmax emitted in the same DVE pass
                nc.vector.tensor_tensor_reduce(
                    out=e_sb[a][:], in0=pe[:], in1=mb[:], scale=1.0,
                    scalar=-3.0e38, op0=Add, op1=Max,
                    accum_out=nm4[:, a : a + 1])
                set_gate("psE", k, touch(e_sb[a]))
                nc.vector.tensor_scalar_mul(
                    nm4[:, NT + a : NT + a + 1], nm4[:, a : a + 1], -1.0)
                # p12 = exp(e_sb - rowmax) over valid-j chunks; z1 for free
                nc.scalar.activation(
                    p12[a][:, :W2], e_sb[a][:, :W2], Exp,
                    bias=nm4[:, NT + a : NT + a + 1],
                    accum_out=z1[:, a : a + 1])
                # rz1 lands in the upper half of z1
                nc.vector.reciprocal(z1[:, NT + a : NT + a + 1],
                                     z1[:, a : a + 1])
                touch(p12[a])
                if a < C1:
                    nc.gpsimd.partition_all_reduce(
                        pm[a][:], e_sb[a][:], 128, bass_isa.ReduceOp.max)

            # column max over valid i (sentinels cancel on subtraction)
            if C1 == 1:
                cm = pm[0]
            else:
                cm = cmp_.tile([128, L], F32, name="cm", tag="cm")
                nc.vector.tensor_max(cm[:], pm[0][:], pm[1][:])
                for a in range(2, C1):
                    nc.vector.tensor_max(cm[:], cm[:], pm[a][:])

            # p21 = exp(e - colmax) with -1e30 bias on padded i
            p21 = [p21p.tile([128, L], FP16, name="p21", tag="p21")
                   for _ in range(C1)]
            for a in range(C1):
                nc.vector.tensor_sub(e_sb[a][:], e_sb[a][:], cm[:])
                nc.scalar.activation(
                    p21[a][:], e_sb[a][:], Exp,
                    bias=m1ct[:, NT * b + a : NT * b + a + 1])
                touch(p21[a])
            return p12, p21, z1

        cp_engs = [nc.gpsimd, nc.vector, nc.scalar]
        cp_i = [0]

        def norm_copy(dst, pt, rz):
            eng = cp_engs[cp_i[0] % 3]
            cp_i[0] += 1
            if eng is nc.scalar:
                eng.activation(dst, pt, Copy, scale=rz)
            else:
                eng.tensor_scalar_mul(dst, pt, rz)

        def emit_stage2(b, head, xb1t, xb2t, last=False):
            C1, C2 = slots[b]
            p12, p21, z1 = head
            touch(xb1t)
            touch(xb2t)

            # stage-2 contraction groups alternate between the psS and psE
            # rings (psE is idle here: e(b+1) has already run), so the
            # norm-copy latency never backpressures the PE
            s2_i = [0]

            def s2_pool():
                s2_i[0] += 1
                return (psS, "psS", 3) if s2_i[0] % 2 else (psE, "psE", 2)

            ysq = []  # deferred output tiles: (a, ys1, ys)

            def x2t_group(a, n, ys):
                pool, tg, nb = s2_pool()
                pt = pool.tile([128, 512], F32, name="s2", tag=tg)
                k = None
                for ai in range(C1):
                    inst = nc.tensor.matmul(
                        pt[:],
                        p21[ai][:, 128 * a : 128 * (a + 1)],
                        xb1t[:, H * ai + 512 * n : H * ai + 512 * (n + 1)],
                        start=(ai == 0), stop=(ai == C1 - 1),
                    )
                    if ai == 0:
                        k = gate(tg, nb, inst)
                norm_copy(ys[:, 512 * n : 512 * (n + 1)], pt[:],
                          rz2[:, a : a + 1])
                set_gate(tg, k, touch(ys[:, 512 * n : 512 * (n + 1)]))

            # transposes of p12 chunks (fp16, exact), interleaved with x2t
            # groups so the psT drain (Pool copy) hides under PE work
            ys_x2 = [o2p.tile([128, H], FP16, name="ys", tag="ys")
                     for _ in range(NT)]
            rz2 = st.tile([128, NT], F32, name="rz2", tag="rz2")
            z2ps = None
            kz2 = None
            p12T = []
            x2q = [(a, n) for a in range(NT) for n in range(2)]
            xi = 0
            for c in range(C2 + 1):
                if c < C2:
                    tp = psT.tile([128, L], FP16, name="psT", tag="psT")
                    k = None
                    for a in range(NT):
                        inst = nc.tensor.matmul(
                            tp[:, 128 * a : 128 * (a + 1)],
                            p12[a][:, 128 * c : 128 * (c + 1)],
                            ident[:], is_transpose=True,
                            start=(a == 0), stop=False,
                        )
                        if a == 0:
                            k = gate("psT", 2, inst)
                    sb = ptp.tile([128, L], FP16, name="p12T", tag="p12T")
                    nc.gpsimd.tensor_copy(sb[:], tp[:])
                    set_gate("psT", k, touch(sb))
                    p12T.append(sb)
                if c == 0:
                    # z2 (rank-1 partition sums of p21) — p21 is long ready
                    # by now, so this never stalls the PE
                    z2ps = psS.tile([128, NT], F32, name="z2ps", tag="psS")
                    for ai in range(C1):
                        for t in range(NT):
                            inst = nc.tensor.matmul(
                                z2ps[:, t : t + 1],
                                p21[ai][:, 128 * t : 128 * (t + 1)],
                                onesh[:], start=(ai == 0 and t == 0),
                                stop=(ai == C1 - 1 and t == NT - 1))
                            if ai == 0 and t == 0:
                                kz2 = gate("psS", 3, inst)
                    nc.vector.reciprocal(rz2[:], z2ps[:])
                    set_gate("psS", kz2, touch(rz2))
                else:
                    # one x2t group between transposes
                    if xi < len(x2q):
                        a, n = x2q[xi]
                        xi += 1
                        x2t_group(a, n, ys_x2[a])

            for a in range(NT):
                ys1 = o2p.tile([128, H], FP16, name="ys1", tag="ys1")
                for n in range(2):
                    pool, tg, nb = s2_pool()
                    pt = pool.tile([128, 512], F32, name="s2", tag=tg)
                    k = None
                    for ci in range(C2):
                        inst = nc.tensor.matmul(
                            pt[:],
                            p12T[ci][:, 128 * a : 128 * (a + 1)],
                            xb2t[:, H * ci + 512 * n : H * ci + 512 * (n + 1)],
                            start=(ci == 0), stop=(ci == C2 - 1),
                        )
                        if ci == 0:
                            k = gate(tg, nb, inst)
                    norm_copy(ys1[:, 512 * n : 512 * (n + 1)], pt[:],
                              z1[:, NT + a : NT + a + 1])
                    set_gate(tg, k, touch(ys1[:, 512 * n : 512 * (n + 1)]))
                if xi < len(x2q):
                    aa, nn = x2q[xi]
                    xi += 1
                    x2t_group(aa, nn, ys_x2[aa])
                if xi < len(x2q):
                    aa, nn = x2q[xi]
                    xi += 1
                    x2t_group(aa, nn, ys_x2[aa])
                rows = slice(128 * a, 128 * (a + 1))
                if last and a == NT - 1:
                    # final tiles: split across all queues to cut the tail
                    bnd = (0, 342, 684, 1024)
                    for qq, q in enumerate((qSP, qACT, qPL)):
                        cs = slice(bnd[qq], bnd[qq + 1])
                        q.dma_start(o1[b, rows, cs], ys1[:, cs])
                        q.dma_start(o2[b, rows, cs], ys_x2[a][:, cs])
                else:
                    qSP.dma_start(o1[b, rows, :], ys1[:])
                    qSP.dma_start(o2[b, rows, :], ys_x2[a][:])

        # ---- software-pipelined batch loop ----
        # PE order: e(b) | stage2(b-1) | transposes+z2(b) | e(b+1) | ...
        nxt = load_batch(0)
        heads = {}
        xbs = {0: (nxt[2], nxt[3])}
        for b in range(BPC):
            xt1t, xt2t, xb1t, xb2t = nxt
            xbs[b] = (xb1t, xb2t)
            heads[b] = emit_head(b, xt1t, xt2t)
            if b + 1 < BPC:
                nxt = load_batch(b + 1)
            if b > 0:
                emit_stage2(b - 1, heads.pop(b - 1), *xbs.pop(b - 1))
        emit_stage2(BPC - 1, heads.pop(BPC - 1), *xbs.pop(BPC - 1),
                    last=True)

    if not nc.is_finalized():
        nc.finalize()
    return nc


def _plan_slots(c1, c2):
    """Partition the B batches into BPC groups of NCORES, minimizing
    sum over groups of (max c1 + max c2).  Returns (slots, assign) where
    assign[core][slot] = original batch index."""
    order = np.argsort(-(c1 + c2), kind="stable")
    groups = [list(order[k * NCORES : (k + 1) * NCORES]) for k in range(BPC)]

    def gcost(g):
        return max(c1[i] for i in g) + max(c2[i] for i in g)

    # local search: swap members between groups while it helps
    improved = True
    it = 0
    while improved and it < 200:
        improved = False
        it += 1
        for ga in range(BPC):
            for gb in range(ga + 1, BPC):
                base = gcost(groups[ga]) + gcost(groups[gb])
                for ia in range(NCORES):
                    for ib in range(NCORES):
                        groups[ga][ia], groups[gb][ib] = (
                            groups[gb][ib], groups[ga][ia])
                        new = gcost(groups[ga]) + gcost(groups[gb])
                        if new < base:
                            base = new
                            improved = True
                        else:
                            groups[ga][ia], groups[gb][ib] = (
                                groups[gb][ib], groups[ga][ia])
    slots = tuple(
        (int(max(c1[i] for i in g)), int(max(c2[i] for i in g)))
        for g in groups)
    assign = [[groups[k][core] for k in range(BPC)]
              for core in range(NCORES)]
    return slots, assign


def kernel(x1_bar, seq_lengths1, x2_bar, seq_lengths2):
    x1_bar = np.ascontiguousarray(x1_bar, dtype=np.float32)
    x2_bar = np.ascontiguousarray(x2_bar, dtype=np.float32)
    sl1 = np.asarray(seq_lengths1).astype(np.int32)
    sl2 = np.asarray(seq_lengths2).astype(np.int32)

    c1 = np.clip((sl1 + 127) // 128, 1, NT).astype(np.int64)
    c2 = np.clip((sl2 + 127) // 128, 1, NT).astype(np.int64)
    slots, assign = _plan_slots(c1, c2)

    xt1f = np.ascontiguousarray(x1_bar.transpose(0, 2, 1))
    xt2f = np.ascontiguousarray(x2_bar.transpose(0, 2, 1))
    xb1f = x1_bar.astype(np.float16)
    xb2f = x2_bar.astype(np.float16)

    ar = np.arange(L, dtype=np.int32)
    pad1 = ar[None, :] >= sl1[:, None]  # [B, L] True on padded i
    pad2 = ar[None, :] >= sl2[:, None]
    m2rowf = np.where(pad2, -SENT, 0.0).astype(np.float32)

    def swz(m, val, idx):
        out = np.where(m[idx], val, 0.0).astype(np.float32)  # [BPC, L]
        return np.ascontiguousarray(
            out.reshape(BPC, NT, 128).transpose(2, 0, 1).reshape(
                128, BPC * NT))

    key = slots
    if key not in _NC_CACHE:
        _NC_CACHE.clear()
        _NC_CACHE[key] = build_nc(slots)
    nc = _NC_CACHE[key]

    in_maps = []
    for core in range(NCORES):
        idx = np.array(assign[core], dtype=np.int64)
        in_maps.append({
            "xt1": np.ascontiguousarray(xt1f[idx]),
            "xt2": np.ascontiguousarray(xt2f[idx]),
            "xb1": np.ascontiguousarray(xb1f[idx]),
            "xb2": np.ascontiguousarray(xb2f[idx]),
            "m2row": np.ascontiguousarray(m2rowf[idx]),
            "m1cs": swz(pad1, -SENT, idx),
            "m1c": swz(pad1, NEG, idx),
        })

    res = run_bass_kernel_spmd(nc, in_maps, core_ids=list(range(NCORES)))

    x1t = np.empty((B, L, H), dtype=np.float32)
    x2t = np.empty((B, L, H), dtype=np.float32)
    for core in range(NCORES):
        r = res.results[core]
        for k in range(BPC):
            bi = assign[core][k]
            x1t[bi] = r["o1"][k].astype(np.float32)
            x2t[bi] = r["o2"][k].astype(np.float32)

    y1 = np.empty((B, L, 4 * H), dtype=np.float32)
    y2 = np.empty((B, L, 4 * H), dtype=np.float32)
    y1[:, :, 0:H] = x1_bar
    y1[:, :, H : 2 * H] = x1t
    y1[:, :, 2 * H : 3 * H] = x1_bar - x1t
    y1[:, :, 3 * H :] = x1_bar * x1t
    y2[:, :, 0:H] = x2_bar
    y2[:, :, H : 2 * H] = x2t
    y2[:, :, 2 * H : 3 * H] = x2_bar - x2t
    y2[:, :, 3 * H :] = x2_bar * x2t
    return y1, y2


# revision 22
# speedup vs baseline: 1.1725x; 1.0041x over previous
"""Trainium2 Bass kernel for nn_LocalInferenceModeling (cross-attention enhance).

Reference computation (per batch b):
    e = x1 @ x2^T                                  [L, L]
    a12 = softmax_j(e + m2[j]);  x1t = a12 @ x2    [L, H]
    a21 = softmax_i(e^T + m1[i]); x2t = a21 @ x1   [L, H]
    y1 = concat([x1, x1t, x1 - x1t, x1 * x1t], -1) [L, 4H]
    y2 = concat([x2, x2t, x2 - x2t, x2 * x2t], -1)

Sharding: batch dim B=32 split across 8 NeuronCores (4 batches/core), no
communication.  The device computes x1_tilde / x2_tilde; the host performs
the final (elementwise) enhance/concat on the exact fp32 inputs.

Device-side design:
  - e is computed once, in fp32 (f32r matmuls at full PE rate), in natural
    [i, j] layout.  A fused DVE tensor_tensor_reduce adds the pad masks
    (bf16-exact sentinel -29952 on padded j columns via a gpsimd partition
    broadcast of the mask row, plus per-partition sentinel on padded i) and
    emits the per-row max in the same pass.
  - p12 = exp(e - rowmax) runs on the Activation engine with a per-partition
    bias, emitting the softmax denominator z1 via accum_out for free.  The
    probabilities are normalized in fp16 (x0.5 DVE cost), transposed on the
    PE at 1 cycle/row (fp16), and contracted against fp16 x2 values.  The
    resulting psum is final (already normalized) and is DMA'd straight from
    PSUM to HBM in fp32.
  - p21 reuses e: column max via gpsimd partition reduces over valid-i tiles
    (the j-sentinel rides along and cancels exactly in fp32 when subtracted),
    exp with a true -1e30 per-partition bias for padded i, z2 via rank-1 PE
    matmuls, normalization folded into the psum->SBUF copies (spread across
    Pool/DVE/Act), fp16 output.
  - Sequence-length sparsity: softmax probabilities of fully-padded 128-row
    chunks are exactly zero, so the stage-2 contractions only run over the
    first C1/C2 chunks.  All 8 cores share one program, so the per-slot
    chunk counts are baked as the max over the cores' batches after a
    host-side assignment that groups batches of similar length; the program
    is rebuilt (and cached) per distinct slot signature.
"""

import sys

import numpy as np

sys.path.insert(0, "/opt/trn_rl_repo")

from contextlib import ExitStack

import concourse.bass as bass
import concourse.bacc as bacc
import concourse.bass_isa as bass_isa
import concourse.mybir as mybir
from concourse import masks
from concourse.bass_utils import run_bass_kernel_spmd
from concourse.tile import TileContext

B, L, H = 32, 512, 1024
NCORES = 8
BPC = B // NCORES  # batches per core
NT = L // 128  # 4 partition tiles per L
HT = H // 128  # 8 partition tiles per H

SENT = np.float32(29952.0)  # bf16-exact sentinel magnitude
NEG = np.float32(-1.0e30)

F32 = mybir.dt.float32
F32R = mybir.dt.float32r
FP16 = mybir.dt.float16

Exp = mybir.ActivationFunctionType.Exp
Copy = mybir.ActivationFunctionType.Copy
Add = mybir.AluOpType.add
Max = mybir.AluOpType.max

_NC_CACHE = {}


def build_nc(slots):
    """slots: tuple of BPC (C1, C2) pairs; C = valid 128-chunk count baked
    into slot k of every core."""
    nc = bacc.Bacc(None, target_bir_lowering=False)
    xt1 = nc.dram_tensor("xt1", [BPC, H, L], F32R, kind="ExternalInput")
    xt2 = nc.dram_tensor("xt2", [BPC, H, L], F32R, kind="ExternalInput")
    xb1 = nc.dram_tensor("xb1", [BPC, L, H], FP16, kind="ExternalInput")
    xb2 = nc.dram_tensor("xb2", [BPC, L, H], FP16, kind="ExternalInput")
    m2row = nc.dram_tensor("m2row", [BPC, L], F32, kind="ExternalInput")
    m1cs = nc.dram_tensor("m1cs", [128, BPC * NT], F32, kind="ExternalInput")
    m1c = nc.dram_tensor("m1c", [128, BPC * NT], F32, kind="ExternalInput")
    o1 = nc.dram_tensor("o1", [BPC, L, H], FP16, kind="ExternalOutput")
    o2 = nc.dram_tensor("o2", [BPC, L, H], FP16, kind="ExternalOutput")

    qSP, qACT, qPL = nc.sync, nc.scalar, nc.gpsimd

    with TileContext(nc) as tc, ExitStack() as ctx:
        from concourse.tile import add_dep_helper

        const = ctx.enter_context(tc.tile_pool(name="const", bufs=1))
        ident = const.tile([128, 128], FP16)
        masks.make_identity(nc, ident[:])
        onesh = const.tile([128, 1], FP16)
        nc.vector.memset(onesh[:], 1.0)
        ones32col = const.tile([128, 1], F32)
        nc.vector.memset(ones32col[:], 1.0)
        ones32 = const.tile([1, 32], F32)
        nc.vector.memset(ones32[:], 1.0)

        xp = ctx.enter_context(tc.tile_pool(name="xp", bufs=2))
        esb = ctx.enter_context(tc.tile_pool(name="esb", bufs=6))
        mbp = ctx.enter_context(tc.tile_pool(name="mbp", bufs=2))
        map_ = ctx.enter_context(tc.tile_pool(name="map", bufs=6))
        pmp = ctx.enter_context(tc.tile_pool(name="pmp", bufs=6))
        cmp_ = ctx.enter_context(tc.tile_pool(name="cmp", bufs=2))
        pp = ctx.enter_context(tc.tile_pool(name="pp", bufs=2 * NT))
        ptp = ctx.enter_context(tc.tile_pool(name="ptp", bufs=2 * NT))
        p21p = ctx.enter_context(tc.tile_pool(name="p21p", bufs=2 * NT))
        st = ctx.enter_context(tc.tile_pool(name="st", bufs=6))
        o2p = ctx.enter_context(tc.tile_pool(name="o2p", bufs=6))
        mrp = ctx.enter_context(tc.tile_pool(name="mrp", bufs=1))
        psE = ctx.enter_context(tc.tile_pool(name="psE", bufs=2, space="PSUM"))
        psT = ctx.enter_context(tc.tile_pool(name="psT", bufs=2, space="PSUM"))
        psS = ctx.enter_context(tc.tile_pool(name="psS", bufs=3, space="PSUM"))
        psScr = ctx.enter_context(
            tc.tile_pool(name="psScr", bufs=1, space="PSUM"))
        scratch = psScr.tile([32, 32], F32, name="scratch", tag="scratch")

        gates = {"psE": [], "psT": [], "psS": []}
        touch_cnt = [0]

        def touch(ap):
            # Tiny PE matmul reading `ap` so the PE engine observes the
            # producer's sem tick; real matmuls then carry at most one sync
            # wait. Rotate over scratch columns so touches don't WAW-chain.
            p = min(ap.shape[0], 32)
            f = min(ap.shape[1], 32)
            if ap.dtype == F32R:
                ap = ap.bitcast(F32)
            oc = onesh if ap.dtype == FP16 else ones32col
            col = touch_cnt[0] % 32
            touch_cnt[0] += 1
            with tc.high_priority(offset=200):
                return nc.tensor.matmul(
                    scratch[0:f, col : col + 1], ap[0:p, 0:f], oc[0:p, 0:1],
                    start=True, stop=True)

        def gate(tag, bufs, first_inst):
            # Order the group's first PE write after the touch that observed
            # the release of the slot it reuses (bufs groups back).
            hist = gates[tag]
            k = len(hist)
            if k >= bufs and hist[k - bufs] is not None:
                add_dep_helper(first_inst.ins, hist[k - bufs].ins, sync=False,
                               reason="psum slot gate")
            hist.append(None)
            return k

        def set_gate(tag, k, tinst):
            gates[tag][k] = tinst

        touch(ident)
        nc.tensor.matmul(scratch[0:32, 0:1], ones32[0:1, :], ones32[0:1, 0:1],
                         start=True, stop=True)

        # ---- static mask loads ----
        m2r = mrp.tile([1, BPC * L], F32, name="m2r", tag="m2r")
        m1cst = mrp.tile([128, BPC * NT], F32, name="m1cst", tag="m1cst")
        m1ct = mrp.tile([128, BPC * NT], F32, name="m1ct", tag="m1ct")
        qACT.dma_start(m2r[:1, :], m2row.rearrange("b l -> (b l)")[None, :])
        qACT.dma_start(m1cst[:], m1cs[:, :])
        qACT.dma_start(m1ct[:], m1c[:, :])

        def load_xt(b, queues=None):
            xt1t = xp.tile([128, HT * L], F32R, name="xt1t", tag="xt1t")
            xt2t = xp.tile([128, HT * L], F32R, name="xt2t", tag="xt2t")
            # transposed e operands, split in half so the first e matmuls
            # can start at half-load.  batch 1's halves borrow the Act queue
            # so the pipeline prologue (two batches of loads, no outputs yet)
            # spreads over three queues instead of two.
            hh = HT // 2
            if queues is None:
                queues = (qSP, qSP, qPL, qPL)
            q1a, q1b, q2a, q2b = queues
            for qa, qb_, t, src in ((q1a, q1b, xt1t, xt1),
                                    (q2a, q2b, xt2t, xt2)):
                qa.dma_start(
                    t[:, : hh * L].rearrange("p (c l) -> p c l", c=hh),
                    src[b, : hh * 128].rearrange("(c p) l -> p c l", p=128))
                qb_.dma_start(
                    t[:, hh * L :].rearrange("p (c l) -> p c l", c=hh),
                    src[b, hh * 128 :].rearrange("(c p) l -> p c l", p=128))
            return xt1t, xt2t

        def load_xb(b, queues=(None, None)):
            C1, C2 = slots[b]
            xb1t = xp.tile([128, NT * H], FP16, name="xb1t", tag="xb1t")
            xb2t = xp.tile([128, NT * H], FP16, name="xb2t", tag="xb2t")
            q1, q2 = queues
            # natural stage-2 values: only the valid chunks are ever read
            (q1 or qACT).dma_start(
                xb1t[:, : C1 * H].rearrange("p (a h) -> p a h", a=C1),
                xb1[b, : C1 * 128].rearrange("(a p) h -> p a h", p=128))
            (q2 or qACT).dma_start(
                xb2t[:, : C2 * H].rearrange("p (a h) -> p a h", a=C2),
                xb2[b, : C2 * 128].rearrange("(a p) h -> p a h", p=128))
            return xb1t, xb2t

        def emit_head(b, xt1t, xt2t):
            """e matmuls + masks/rowmax + p12 (normalized fp16) + colmax cm
            + p21 + z2.  Returns what stage 2 needs."""
            C1, C2 = slots[b]
            touch(xt1t)
            touch(xt2t)

            m2b = mbp.tile([128, L], F32, name="m2b", tag="m2b")
            nc.gpsimd.partition_broadcast(
                m2b[:], m2r[0:1, L * b : L * (b + 1)], 128)
            mba = []
            for a in range(C1):
                t = map_.tile([128, L], F32, name="mba", tag="mba")
                nc.gpsimd.tensor_scalar_add(
                    t[:], m2b[:], m1cst[:, NT * b + a : NT * b + a + 1])
                mba.append(t)

            nm4 = st.tile([128, 2 * NT], F32, name="nm4", tag="nm4")
            z1 = st.tile([128, 2 * NT], F32, name="z1", tag="z1")
            e_sb = [esb.tile([128, L], F32, name="e_sb", tag="e_sb")
                    for _ in range(NT)]
            p12 = [pp.tile([128, L], FP16, name="p12", tag="p12")
                   for _ in range(NT)]
            pm = [pmp.tile([128, L], F32, name="pm", tag="pm")
                  for _ in range(C1)]
            W2 = C2 * 128
            for a in range(NT):
                pe = psE.tile([128, L], F32, name="psE", tag="psE")
                k = None
                for c in range(HT):
                    inst = nc.tensor.matmul(
                        pe[:],
                        xt1t[:, L * c + 128 * a : L * c + 128 * (a + 1)],
                        xt2t[:, L * c : L * (c + 1)],
                        start=(c == 0),
                        stop=(c == HT - 1),
                    )
                    if c == 0:
                        k = gate("psE", 2, inst)
                mb = mba[a] if a < C1 else m2b
                # e_sb = e + masks; rowmax emitted in the same DVE pass
                nc.vector.tensor_tensor_reduce(
                    out=e_sb[a][:], in0=pe[:], in1=mb[:], scale=1.0,
                    scalar=-3.0e38, op0=Add, op1=Max,
                    accum_out=nm4[:, a : a + 1])
                set_gate("psE", k, touch(e_sb[a]))
                nc.vector.tensor_scalar_mul(
                    nm4[:, NT + a : NT + a + 1], nm4[:, a : a + 1], -1.0)
                # p12 = exp(e_sb - rowmax) over valid-j chunks; z1 for free
                nc.scalar.activation(
                    p12[a][:, :W2], e_sb[a][:, :W2], Exp,
                    bias=nm4[:, NT + a : NT + a + 1],
                    accum_out=z1[:, a : a + 1])
                # rz1 lands in the upper half of z1
                nc.vector.reciprocal(z1[:, NT + a : NT + a + 1],
                                     z1[:, a : a + 1])
                touch(p12[a])
                if a < C1:
                    nc.gpsimd.partition_all_reduce(
                        pm[a][:], e_sb[a][:], 128, bass_isa.ReduceOp.max)

            # column max over valid i (sentinels cancel on subtraction)
            if C1 == 1:
                cm = pm[0]
            else:
                cm = cmp_.tile([128, L], F32, name="cm", tag="cm")
                nc.vector.tensor_max(cm[:], pm[0][:], pm[1][:])
                for a in range(2, C1):
                    nc.vector.tensor_max(cm[:], cm[:], pm[a][:])

            # p21 = exp(e - colmax) with -1e30 bias on padded i
            p21 = [p21p.tile([128, L], FP16, name="p21", tag="p21")
                   for _ in range(C1)]
            for a in range(C1):
                nc.vector.tensor_sub(e_sb[a][:], e_sb[a][:], cm[:])
                nc.scalar.activation(
                    p21[a][:], e_sb[a][:], Exp,
                    bias=m1ct[:, NT * b + a : NT * b + a + 1])
                touch(p21[a])
            return p12, p21, z1

        cp_engs = [nc.gpsimd, nc.vector, nc.scalar]
        cp_i = [0]

        def norm_copy(dst, pt, rz):
            eng = cp_engs[cp_i[0] % 3]
            cp_i[0] += 1
            if eng is nc.scalar:
                eng.activation(dst, pt, Copy, scale=rz)
            else:
                eng.tensor_scalar_mul(dst, pt, rz)

        def emit_stage2(b, head, xb1t, xb2t, last=False):
            C1, C2 = slots[b]
            p12, p21, z1 = head
            touch(xb1t)
            touch(xb2t)

            # stage-2 contraction groups alternate between the psS and psE
            # rings (psE is idle here: e(b+1) has already run), so the
            # norm-copy latency never backpressures the PE
            s2_i = [0]

            def s2_pool():
                s2_i[0] += 1
                return (psS, "psS", 3) if s2_i[0] % 2 else (psE, "psE", 2)

            ysq = []  # deferred output tiles: (a, ys1, ys)

            def x2t_group(a, n, ys):
                pool, tg, nb = s2_pool()
                pt = pool.tile([128, 512], F32, name="s2", tag=tg)
                k = None
                for ai in range(C1):
                    inst = nc.tensor.matmul(
                        pt[:],
                        p21[ai][:, 128 * a : 128 * (a + 1)],
                        xb1t[:, H * ai + 512 * n : H * ai + 512 * (n + 1)],
                        start=(ai == 0), stop=(ai == C1 - 1),
                    )
                    if ai == 0:
                        k = gate(tg, nb, inst)
                norm_copy(ys[:, 512 * n : 512 * (n + 1)], pt[:],
                          rz2[:, a : a + 1])
                set_gate(tg, k, touch(ys[:, 512 * n : 512 * (n + 1)]))

            # transposes of p12 chunks (fp16, exact), interleaved with x2t
            # groups so the psT drain (Pool copy) hides under PE work
            ys_x2 = [o2p.tile([128, H], FP16, name="ys", tag="ys")
                     for _ in range(NT)]
            rz2 = st.tile([128, NT], F32, name="rz2", tag="rz2")
            z2ps = None
            kz2 = None
            p12T = []
            x2q = [(a, n) for a in range(NT) for n in range(2)]
            xi = 0
            for c in range(C2 + 1):
                if c < C2:
                    tp = psT.tile([128, L], FP16, name="psT", tag="psT")
                    k = None
                    for a in range(NT):
                        inst = nc.tensor.matmul(
                            tp[:, 128 * a : 128 * (a + 1)],
                            p12[a][:, 128 * c : 128 * (c + 1)],
                            ident[:], is_transpose=True,
                            start=(a == 0), stop=False,
                        )
                        if a == 0:
                            k = gate("psT", 2, inst)
                    sb = ptp.tile([128, L], FP16, name="p12T", tag="p12T")
                    nc.gpsimd.tensor_copy(sb[:], tp[:])
                    set_gate("psT", k, touch(sb))
                    p12T.append(sb)
                if c == 0:
                    # z2 (rank-1 partition sums of p21) — p21 is long ready
                    # by now, so this never stalls the PE
                    z2ps = psS.tile([128, NT], F32, name="z2ps", tag="psS")
                    for ai in range(C1):
                        for t in range(NT):
                            inst = nc.tensor.matmul(
                                z2ps[:, t : t + 1],
                                p21[ai][:, 128 * t : 128 * (t + 1)],
                                onesh[:], start=(ai == 0 and t == 0),
                                stop=(ai == C1 - 1 and t == NT - 1))
                            if ai == 0 and t == 0:
                                kz2 = gate("psS", 3, inst)
                    nc.vector.reciprocal(rz2[:], z2ps[:])
                    set_gate("psS", kz2, touch(rz2))
                else:
                    # one x2t group between transposes
                    if xi < len(x2q):
                        a, n = x2q[xi]
                        xi += 1
                        x2t_group(a, n, ys_x2[a])

            for a in range(NT):
                ys1 = o2p.tile([128, H], FP16, name="ys1", tag="ys1")
                for n in range(2):
                    pool, tg, nb = s2_pool()
                    pt = pool.tile([128, 512], F32, name="s2", tag=tg)
                    k = None
                    for ci in range(C2):
                        inst = nc.tensor.matmul(
                            pt[:],
                            p12T[ci][:, 128 * a : 128 * (a + 1)],
                            xb2t[:, H * ci + 512 * n : H * ci + 512 * (n + 1)],
                            start=(ci == 0), stop=(ci == C2 - 1),
                        )
                        if ci == 0:
                            k = gate(tg, nb, inst)
                    norm_copy(ys1[:, 512 * n : 512 * (n + 1)], pt[:],
                              z1[:, NT + a : NT + a + 1])
                    set_gate(tg, k, touch(ys1[:, 512 * n : 512 * (n + 1)]))
                if xi < len(x2q):
                    aa, nn = x2q[xi]
                    xi += 1
                    x2t_group(aa, nn, ys_x2[aa])
                if xi < len(x2q):
                    aa, nn = x2q[xi]
                    xi += 1
                    x2t_group(aa, nn, ys_x2[aa])
                rows = slice(128 * a, 128 * (a + 1))
                if last and a == NT - 1:
                    # final tiles: split across all queues to cut the tail
                    bnd = (0, 342, 684, 1024)
                    for qq, q in enumerate((qSP, qACT, qPL)):
                        cs = slice(bnd[qq], bnd[qq + 1])
                        q.dma_start(o1[b, rows, cs], ys1[:, cs])
                        q.dma_start(o2[b, rows, cs], ys_x2[a][:, cs])
                else:
                    qSP.dma_start(o1[b, rows, :], ys1[:])
                    qSP.dma_start(o2[b, rows, :], ys_x2[a][:])

        # ---- software-pipelined batch loop ----
        # PE order: e(0) | e(1) | T/z2/s2(0) | e(2) | T/z2/s2(1) | ...
        # Prologue loads are hand-spread: both batches' xt over all three
        # queues (no outputs compete yet), xb(0) on SP/Pool after xt.
        xts = {0: load_xt(0)}
        xts[1] = load_xt(1, queues=(qACT, qSP, qPL, qACT))
        xbs = {0: load_xb(0, queues=(qSP, qPL))}
        heads = {}
        for b in range(BPC):
            heads[b] = emit_head(b, *xts.pop(b))
            if b + 1 < BPC:
                if b + 1 not in xts:
                    xts[b + 1] = load_xt(b + 1)
                xbs[b + 1] = load_xb(b + 1)
            if b > 0:
                emit_stage2(b - 1, heads.pop(b - 1), *xbs.pop(b - 1))
        emit_stage2(BPC - 1, heads.pop(BPC - 1), *xbs.pop(BPC - 1),
                    last=True)

    if not nc.is_finalized():
        nc.finalize()
    return nc


def _plan_slots(c1, c2):
    """Partition the B batches into BPC groups of NCORES, minimizing
    sum over groups of (max c1 + max c2).  Returns (slots, assign) where
    assign[core][slot] = original batch index."""
    order = np.argsort(-(c1 + c2), kind="stable")
    groups = [list(order[k * NCORES : (k + 1) * NCORES]) for k in range(BPC)]

    def gcost(g):
        return max(c1[i] for i in g) + max(c2[i] for i in g)

    # local search: swap members between groups while it helps
    improved = True
    it = 0
    while improved and it < 200:
        improved = False
        it += 1
        for ga in range(BPC):
            for gb in range(ga + 1, BPC):
                base = gcost(groups[ga]) + gcost(groups[gb])
                for ia in range(NCORES):
                    for ib in range(NCORES):
                        groups[ga][ia], groups[gb][ib] = (
                            groups[gb][ib], groups[ga][ia])
                        new = gcost(groups[ga]) + gcost(groups[gb])
                        if new < base:
                            base = new
                            improved = True
                        else:
                            groups[ga][ia], groups[gb][ib] = (
                                groups[gb][ib], groups[ga][ia])
    slots = tuple(
        (int(max(c1[i] for i in g)), int(max(c2[i] for i in g)))
        for g in groups)
    assign = [[groups[k][core] for k in range(BPC)]
              for core in range(NCORES)]
    return slots, assign


def kernel(x1_bar, seq_lengths1, x2_bar, seq_lengths2):
    x1_bar = np.ascontiguousarray(x1_bar, dtype=np.float32)
    x2_bar = np.ascontiguousarray(x2_bar, dtype=np.float32)
    sl1 = np.asarray(seq_lengths1).astype(np.int32)
    sl2 = np.asarray(seq_lengths2).astype(np.int32)

    c1 = np.clip((sl1 + 127) // 128, 1, NT).astype(np.int64)
    c2 = np.clip((sl2 + 127) // 128, 1, NT).astype(np.int64)
    slots, assign = _plan_slots(c1, c2)

    xt1f = np.ascontiguousarray(x1_bar.transpose(0, 2, 1))
    xt2f = np.ascontiguousarray(x2_bar.transpose(0, 2, 1))
    xb1f = x1_bar.astype(np.float16)
    xb2f = x2_bar.astype(np.float16)

    ar = np.arange(L, dtype=np.int32)
    pad1 = ar[None, :] >= sl1[:, None]  # [B, L] True on padded i
    pad2 = ar[None, :] >= sl2[:, None]
    m2rowf = np.where(pad2, -SENT, 0.0).astype(np.float32)

    def swz(m, val, idx):
        out = np.where(m[idx], val, 0.0).astype(np.float32)  # [BPC, L]
        return np.ascontiguousarray(
            out.reshape(BPC, NT, 128).transpose(2, 0, 1).reshape(
                128, BPC * NT))

    key = slots
    if key not in _NC_CACHE:
        _NC_CACHE.clear()
        _NC_CACHE[key] = build_nc(slots)
    nc = _NC_CACHE[key]

    in_maps = []
    for core in range(NCORES):
        idx = np.array(assign[core], dtype=np.int64)
        in_maps.append({
            "xt1": np.ascontiguousarray(xt1f[idx]),
            "xt2": np.ascontiguousarray(xt2f[idx]),
            "xb1": np.ascontiguousarray(xb1f[idx]),
            "xb2": np.ascontiguousarray(xb2f[idx]),
            "m2row": np.ascontiguousarray(m2rowf[idx]),
            "m1cs": swz(pad1, -SENT, idx),
            "m1c": swz(pad1, NEG, idx),
        })

    res = run_bass_kernel_spmd(nc, in_maps, core_ids=list(range(NCORES)))

    x1t = np.empty((B, L, H), dtype=np.float32)
    x2t = np.empty((B, L, H), dtype=np.float32)
    for core in range(NCORES):
        r = res.results[core]
        for k in range(BPC):
            bi = assign[core][k]
            x1t[bi] = r["o1"][k].astype(np.float32)
            x2t[bi] = r["o2"][k].astype(np.float32)

    y1 = np.empty((B, L, 4 * H), dtype=np.float32)
    y2 = np.empty((B, L, 4 * H), dtype=np.float32)
    y1[:, :, 0:H] = x1_bar
    y1[:, :, H : 2 * H] = x1t
    y1[:, :, 2 * H : 3 * H] = x1_bar - x1t
    y1[:, :, 3 * H :] = x1_bar * x1t
    y2[:, :, 0:H] = x2_bar
    y2[:, :, H : 2 * H] = x2t
    y2[:, :, 2 * H : 3 * H] = x2_bar - x2t
    y2[:, :, 3 * H :] = x2_bar * x2t
    return y1, y2


# revision 33
# speedup vs baseline: 1.2152x; 1.0363x over previous
"""Trainium2 Bass kernel for nn_LocalInferenceModeling (cross-attention enhance).

Reference computation (per batch b):
    e = x1 @ x2^T                                  [L, L]
    a12 = softmax_j(e + m2[j]);  x1t = a12 @ x2    [L, H]
    a21 = softmax_i(e^T + m1[i]); x2t = a21 @ x1   [L, H]
    y1 = concat([x1, x1t, x1 - x1t, x1 * x1t], -1) [L, 4H]
    y2 = concat([x2, x2t, x2 - x2t, x2 * x2t], -1)

Sharding: batch dim B=32 split across 8 NeuronCores (4 batches/core), no
communication.  The device computes x1_tilde / x2_tilde; the host performs
the final (elementwise) enhance/concat on the exact fp32 inputs.

Device-side design:
  - e is computed once, in fp32 (f32r matmuls at full PE rate), in natural
    [i, j] layout.  A fused DVE tensor_tensor_reduce adds the pad masks
    (bf16-exact sentinel -29952 on padded j columns via a gpsimd partition
    broadcast of the mask row, plus per-partition sentinel on padded i) and
    emits the per-row max in the same pass.
  - p12 = exp(e - rowmax) runs on the Activation engine with a per-partition
    bias, emitting the softmax denominator z1 via accum_out for free.  The
    probabilities are normalized in fp16 (x0.5 DVE cost), transposed on the
    PE at 1 cycle/row (fp16), and contracted against fp16 x2 values.  The
    resulting psum is final (already normalized) and is DMA'd straight from
    PSUM to HBM in fp32.
  - p21 reuses e: column max via gpsimd partition reduces over valid-i tiles
    (the j-sentinel rides along and cancels exactly in fp32 when subtracted),
    exp with a true -1e30 per-partition bias for padded i, z2 via rank-1 PE
    matmuls, normalization folded into the psum->SBUF copies (spread across
    Pool/DVE/Act), fp16 output.
  - Sequence-length sparsity: softmax probabilities of fully-padded 128-row
    chunks are exactly zero, so the stage-2 contractions only run over the
    first C1/C2 chunks.  All 8 cores share one program, so the per-slot
    chunk counts are baked as the max over the cores' batches after a
    host-side assignment that groups batches of similar length; the program
    is rebuilt (and cached) per distinct slot signature.
"""

import sys

import numpy as np

sys.path.insert(0, "/opt/trn_rl_repo")

from contextlib import ExitStack

import concourse.bass as bass
import concourse.bacc as bacc
import concourse.bass_isa as bass_isa
import concourse.mybir as mybir
from concourse import masks
from concourse.bass_utils import run_bass_kernel_spmd
from concourse.tile import TileContext

B, L, H = 32, 512, 1024
NCORES = 8
BPC = B // NCORES  # batches per core
NT = L // 128  # 4 partition tiles per L
HT = H // 128  # 8 partition tiles per H

SENT = np.float32(29952.0)  # bf16-exact sentinel magnitude
NEG = np.float32(-1.0e30)

F32 = mybir.dt.float32
F32R = mybir.dt.float32r
FP16 = mybir.dt.float16

Exp = mybir.ActivationFunctionType.Exp
Copy = mybir.ActivationFunctionType.Copy
Add = mybir.AluOpType.add
Max = mybir.AluOpType.max

_NC_CACHE = {}


def build_nc(slots):
    """slots: tuple of BPC (C1, C2) pairs; C = valid 128-chunk count baked
    into slot k of every core."""
    nc = bacc.Bacc(None, target_bir_lowering=False)
    xt1 = nc.dram_tensor("xt1", [BPC, H, L], F32R, kind="ExternalInput")
    xt2 = nc.dram_tensor("xt2", [BPC, H, L], F32R, kind="ExternalInput")
    xb1 = nc.dram_tensor("xb1", [BPC, L, H], FP16, kind="ExternalInput")
    xb2 = nc.dram_tensor("xb2", [BPC, L, H], FP16, kind="ExternalInput")
    m2row = nc.dram_tensor("m2row", [BPC, L], FP16, kind="ExternalInput")
    m1cs = nc.dram_tensor("m1cs", [128, BPC * NT], F32, kind="ExternalInput")
    m1c = nc.dram_tensor("m1c", [128, BPC * NT], F32, kind="ExternalInput")
    o1 = nc.dram_tensor("o1", [BPC, L, H], FP16, kind="ExternalOutput")
    o2 = nc.dram_tensor("o2", [BPC, L, H], FP16, kind="ExternalOutput")

    qSP, qACT, qPL = nc.sync, nc.scalar, nc.gpsimd

    with TileContext(nc) as tc, ExitStack() as ctx:
        from concourse.tile import add_dep_helper

        const = ctx.enter_context(tc.tile_pool(name="const", bufs=1))
        ident = const.tile([128, 128], FP16)
        masks.make_identity(nc, ident[:])
        onesh = const.tile([128, 1], FP16)
        nc.vector.memset(onesh[:], 1.0)
        onesrow = const.tile([1, 128], FP16)
        nc.vector.memset(onesrow[:], 1.0)
        ones32col = const.tile([128, 1], F32)
        nc.vector.memset(ones32col[:], 1.0)
        ones32 = const.tile([1, 32], F32)
        nc.vector.memset(ones32[:], 1.0)

        xp = ctx.enter_context(tc.tile_pool(name="xp", bufs=2))
        esb = ctx.enter_context(tc.tile_pool(name="esb", bufs=6))
        pmp = ctx.enter_context(tc.tile_pool(name="pmp", bufs=6))
        cmp_ = ctx.enter_context(tc.tile_pool(name="cmp", bufs=2))
        pp = ctx.enter_context(tc.tile_pool(name="pp", bufs=2 * NT))
        ptp = ctx.enter_context(tc.tile_pool(name="ptp", bufs=2 * NT))
        p21p = ctx.enter_context(tc.tile_pool(name="p21p", bufs=2 * NT))
        st = ctx.enter_context(tc.tile_pool(name="st", bufs=6))
        o2p = ctx.enter_context(tc.tile_pool(name="o2p", bufs=8))
        mrp = ctx.enter_context(tc.tile_pool(name="mrp", bufs=1))
        psE = ctx.enter_context(tc.tile_pool(name="psE", bufs=2, space="PSUM"))
        psT = ctx.enter_context(tc.tile_pool(name="psT", bufs=2, space="PSUM"))
        psS = ctx.enter_context(tc.tile_pool(name="psS", bufs=3, space="PSUM"))
        psScr = ctx.enter_context(
            tc.tile_pool(name="psScr", bufs=1, space="PSUM"))
        scratch = psScr.tile([32, 32], F32, name="scratch", tag="scratch")

        gates = {"psE": [], "psT": [], "psS": []}
        touch_cnt = [0]

        def touch(ap):
            # Tiny PE matmul reading `ap` so the PE engine observes the
            # producer's sem tick; real matmuls then carry at most one sync
            # wait. Rotate over scratch columns so touches don't WAW-chain.
            p = min(ap.shape[0], 32)
            f = min(ap.shape[1], 32)
            if ap.dtype == F32R:
                ap = ap.bitcast(F32)
            oc = onesh if ap.dtype == FP16 else ones32col
            col = touch_cnt[0] % 32
            touch_cnt[0] += 1
            with tc.high_priority(offset=200):
                return nc.tensor.matmul(
                    scratch[0:f, col : col + 1], ap[0:p, 0:f], oc[0:p, 0:1],
                    start=True, stop=True)

        def gate(tag, bufs, first_inst):
            # Order the group's first PE write after the touch that observed
            # the release of the slot it reuses (bufs groups back).
            hist = gates[tag]
            k = len(hist)
            if k >= bufs and hist[k - bufs] is not None:
                add_dep_helper(first_inst.ins, hist[k - bufs].ins, sync=False,
                               reason="psum slot gate")
            hist.append(None)
            return k

        def set_gate(tag, k, tinst):
            gates[tag][k] = tinst

        touch(ident)
        nc.tensor.matmul(scratch[0:32, 0:1], ones32[0:1, :], ones32[0:1, 0:1],
                         start=True, stop=True)

        # ---- static mask loads ----
        m2r = mrp.tile([1, BPC * L], FP16, name="m2r", tag="m2r")
        m1cst = mrp.tile([128, BPC * NT], F32, name="m1cst", tag="m1cst")
        m1ct = mrp.tile([128, BPC * NT], F32, name="m1ct", tag="m1ct")
        qACT.dma_start(m2r[:1, :], m2row.rearrange("b l -> (b l)")[None, :])
        qACT.dma_start(m1cst[:], m1cs[:, :])
        qACT.dma_start(m1ct[:], m1c[:, :])

        def load_xt(b, queues=None):
            xt1t = xp.tile([128, HT * L], F32R, name="xt1t", tag="xt1t")
            xt2t = xp.tile([128, HT * L], F32R, name="xt2t", tag="xt2t")
            # transposed e operands, split in half so the first e matmuls
            # can start at half-load.  batch 1's halves borrow the Act queue
            # so the pipeline prologue (two batches of loads, no outputs yet)
            # spreads over three queues instead of two.
            hh = HT // 2
            if queues is None:
                queues = (qSP, qSP, qSP, qSP)
            q1a, q1b, q2a, q2b = queues
            for qa, qb_, t, src in ((q1a, q1b, xt1t, xt1),
                                    (q2a, q2b, xt2t, xt2)):
                qa.dma_start(
                    t[:, : hh * L].rearrange("p (c l) -> p c l", c=hh),
                    src[b, : hh * 128].rearrange("(c p) l -> p c l", p=128))
                qb_.dma_start(
                    t[:, hh * L :].rearrange("p (c l) -> p c l", c=hh),
                    src[b, hh * 128 :].rearrange("(c p) l -> p c l", p=128))
            return xt1t, xt2t

        def load_xb(b, queues=(None, None)):
            C1, C2 = slots[b]
            xb1t = xp.tile([128, NT * H], FP16, name="xb1t", tag="xb1t")
            xb2t = xp.tile([128, NT * H], FP16, name="xb2t", tag="xb2t")
            q1, q2 = queues
            # natural stage-2 values: only the valid chunks are ever read
            (q1 or qACT).dma_start(
                xb1t[:, : C1 * H].rearrange("p (a h) -> p a h", a=C1),
                xb1[b, : C1 * 128].rearrange("(a p) h -> p a h", p=128))
            (q2 or qACT).dma_start(
                xb2t[:, : C2 * H].rearrange("p (a h) -> p a h", a=C2),
                xb2[b, : C2 * 128].rearrange("(a p) h -> p a h", p=128))
            return xb1t, xb2t

        def emit_head(b, xt1t, xt2t):
            """e matmuls + masks/rowmax + p12 (fp16) + colmax cm + p21.
            Returns what stage 2 needs."""
            C1, C2 = slots[b]
            touch(xt1t)
            touch(xt2t)
            m2row_b = m2r[0:1, L * b : L * (b + 1)]

            nm4 = st.tile([128, NT], F32, name="nm4", tag="nm4")
            z1 = st.tile([128, 2 * NT], F32, name="z1", tag="z1")
            e_sb = [esb.tile([128, L], F32, name="e_sb", tag="e_sb")
                    for _ in range(NT)]
            p12 = [pp.tile([128, L], FP16, name="p12", tag="p12")
                   for _ in range(NT)]
            pm = [pmp.tile([128, L], F32, name="pm", tag="pm")
                  for _ in range(C1)]
            W2 = C2 * 128
            for a in range(NT):
                pe = psE.tile([128, L], F32, name="psE", tag="psE")
                k = None
                for c in range(HT):
                    inst = nc.tensor.matmul(
                        pe[:],
                        xt1t[:, L * c + 128 * a : L * c + 128 * (a + 1)],
                        xt2t[:, L * c : L * (c + 1)],
                        start=(c == 0),
                        stop=False,
                    )
                    if c == 0:
                        k = gate("psE", 2, inst)
                # j-pad sentinel rank-1 (uniform -SENT on padded j columns)
                nc.tensor.matmul(pe[:], onesrow[0:1, :], m2row_b,
                                 start=False, stop=True)
                # e_sb = e + m2sent + m1sent[i] (per-partition, valid tiles
                # only); negated rowmax via the fused reduce
                sc = (m1cst[:, NT * b + a : NT * b + a + 1]
                      if a < C1 else 0.0)
                nc.vector.tensor_scalar_add(e_sb[a][:], pe[:], sc)
                set_gate("psE", k, touch(e_sb[a]))
                nc.vector.reduce_max(nm4[:, a : a + 1], e_sb[a][:],
                                     axis=mybir.AxisListType.X, negate=True)
                # p12 = exp(e_sb - rowmax) over valid-j chunks; z1 for free
                nc.scalar.activation(
                    p12[a][:, :W2], e_sb[a][:, :W2], Exp,
                    bias=nm4[:, a : a + 1],
                    accum_out=z1[:, a : a + 1])
                # rz1 lands in the upper half of z1
                nc.vector.reciprocal(z1[:, NT + a : NT + a + 1],
                                     z1[:, a : a + 1])
                touch(p12[a])
                if a < C1:
                    nc.gpsimd.partition_all_reduce(
                        pm[a][:], e_sb[a][:], 128, bass_isa.ReduceOp.max)

            # column max over valid i (sentinels cancel on subtraction)
            if C1 == 1:
                cm = pm[0]
            else:
                cm = cmp_.tile([128, L], F32, name="cm", tag="cm")
                nc.vector.tensor_max(cm[:], pm[0][:], pm[1][:])
                for a in range(2, C1):
                    nc.vector.tensor_max(cm[:], cm[:], pm[a][:])

            # p21 = exp(e - colmax) with -1e30 bias on padded i
            p21 = [p21p.tile([128, L], FP16, name="p21", tag="p21")
                   for _ in range(C1)]
            for a in range(C1):
                nc.vector.tensor_sub(e_sb[a][:], e_sb[a][:], cm[:])
                nc.scalar.activation(
                    p21[a][:], e_sb[a][:], Exp,
                    bias=m1ct[:, NT * b + a : NT * b + a + 1])
                touch(p21[a])
            return p12, p21, z1

        cp_engs = [nc.gpsimd, nc.vector, nc.gpsimd]
        cp_i = [0]

        def norm_copy(dst, pt, rz):
            eng = cp_engs[cp_i[0] % 3]
            cp_i[0] += 1
            if eng is nc.scalar:
                eng.activation(dst, pt, Copy, scale=rz)
            else:
                eng.tensor_scalar_mul(dst, pt, rz)

        def emit_stage2(b, head, xb1t, xb2t, last=False):
            C1, C2 = slots[b]
            p12, p21, z1 = head
            touch(xb1t)
            touch(xb2t)

            # stage-2 contraction groups alternate between the psS and psE
            # rings (psE is idle here: e(b+1) has already run), so the
            # norm-copy latency never backpressures the PE
            s2_i = [0]

            def s2_pool():
                s2_i[0] += 1
                return (psS, "psS", 3) if s2_i[0] % 2 else (psE, "psE", 2)

            ysq = []  # deferred output tiles: (a, ys1, ys)

            def x2t_group(a, n, ys):
                pool, tg, nb = s2_pool()
                pt = pool.tile([128, 512], F32, name="s2", tag=tg)
                k = None
                for ai in range(C1):
                    inst = nc.tensor.matmul(
                        pt[:],
                        p21[ai][:, 128 * a : 128 * (a + 1)],
                        xb1t[:, H * ai + 512 * n : H * ai + 512 * (n + 1)],
                        start=(ai == 0), stop=(ai == C1 - 1),
                    )
                    if ai == 0:
                        k = gate(tg, nb, inst)
                norm_copy(ys[:, 512 * n : 512 * (n + 1)], pt[:],
                          rz2[:, a : a + 1])
                set_gate(tg, k, touch(ys[:, 512 * n : 512 * (n + 1)]))

            # transposes of p12 chunks (fp16, exact), interleaved with x2t
            # groups so the psT drain (Pool copy) hides under PE work
            ys_x2 = [o2p.tile([128, H], FP16, name="ys", tag="ys")
                     for _ in range(NT)]
            rz2 = st.tile([128, NT], F32, name="rz2", tag="rz2")
            z2ps = None
            kz2 = None
            p12T = []
            x2q = [(a, n) for a in range(NT) for n in range(2)]
            xi = 0
            for c in range(C2 + 1):
                if c < C2:
                    tp = psT.tile([128, L], FP16, name="psT", tag="psT")
                    k = None
                    for a in range(NT):
                        inst = nc.tensor.matmul(
                            tp[:, 128 * a : 128 * (a + 1)],
                            p12[a][:, 128 * c : 128 * (c + 1)],
                            ident[:], is_transpose=True,
                            start=(a == 0), stop=False,
                        )
                        if a == 0:
                            k = gate("psT", 2, inst)
                    sb = ptp.tile([128, L], FP16, name="p12T", tag="p12T")
                    nc.gpsimd.tensor_copy(sb[:], tp[:])
                    set_gate("psT", k, touch(sb))
                    p12T.append(sb)
                if c == 0:
                    # z2 (rank-1 partition sums of p21) — p21 is long ready
                    # by now, so this never stalls the PE
                    z2ps = psS.tile([128, NT], F32, name="z2ps", tag="psS")
                    for ai in range(C1):
                        for t in range(NT):
                            inst = nc.tensor.matmul(
                                z2ps[:, t : t + 1],
                                p21[ai][:, 128 * t : 128 * (t + 1)],
                                onesh[:], start=(ai == 0 and t == 0),
                                stop=(ai == C1 - 1 and t == NT - 1))
                            if ai == 0 and t == 0:
                                kz2 = gate("psS", 3, inst)
                    nc.vector.reciprocal(rz2[:], z2ps[:])
                    set_gate("psS", kz2, touch(rz2))
                else:
                    # one x2t group between transposes
                    if xi < len(x2q):
                        a, n = x2q[xi]
                        xi += 1
                        x2t_group(a, n, ys_x2[a])

            for a in range(NT):
                ys1 = o2p.tile([128, H], FP16, name="ys1", tag="ys1")
                for n in range(2):
                    pool, tg, nb = s2_pool()
                    pt = pool.tile([128, 512], F32, name="s2", tag=tg)
                    k = None
                    for ci in range(C2):
                        inst = nc.tensor.matmul(
                            pt[:],
                            p12T[ci][:, 128 * a : 128 * (a + 1)],
                            xb2t[:, H * ci + 512 * n : H * ci + 512 * (n + 1)],
                            start=(ci == 0), stop=(ci == C2 - 1),
                        )
                        if ci == 0:
                            k = gate(tg, nb, inst)
                    norm_copy(ys1[:, 512 * n : 512 * (n + 1)], pt[:],
                              z1[:, NT + a : NT + a + 1])
                    set_gate(tg, k, touch(ys1[:, 512 * n : 512 * (n + 1)]))
                if xi < len(x2q):
                    aa, nn = x2q[xi]
                    xi += 1
                    x2t_group(aa, nn, ys_x2[aa])
                if xi < len(x2q):
                    aa, nn = x2q[xi]
                    xi += 1
                    x2t_group(aa, nn, ys_x2[aa])
                rows = slice(128 * a, 128 * (a + 1))
                if last and a == NT - 1:
                    # final tiles: split across all queues to cut the tail
                    bnd = (0, 342, 684, 1024)
                    for qq, q in enumerate((qSP, qACT, qPL)):
                        cs = slice(bnd[qq], bnd[qq + 1])
                        q.dma_start(o1[b, rows, cs], ys1[:, cs])
                        q.dma_start(o2[b, rows, cs], ys_x2[a][:, cs])
                else:
                    qACT.dma_start(o1[b, rows, :], ys1[:])
                    qACT.dma_start(o2[b, rows, :], ys_x2[a][:])

        # ---- software-pipelined batch loop ----
        # PE order: e(0) | e(1) | T/z2/s2(0) | e(2) | T/z2/s2(1) | ...
        # Prologue loads are hand-spread over all three queues (no outputs
        # compete yet); steady state keeps xt on SP (pure DMA queue), xb and
        # outputs on Act, and the Pool queue free for latency-critical
        # copies.
        xts = {0: load_xt(0, queues=(qSP, qSP, qPL, qPL))}
        xts[1] = load_xt(1, queues=(qACT, qSP, qACT, qPL))
        xbs = {0: load_xb(0, queues=(qPL, qPL))}
        heads = {}
        for b in range(BPC):
            heads[b] = emit_head(b, *xts.pop(b))
            if b + 1 < BPC:
                if b + 1 not in xts:
                    xts[b + 1] = load_xt(b + 1)
                xbs[b + 1] = load_xb(b + 1)
            if b > 0:
                emit_stage2(b - 1, heads.pop(b - 1), *xbs.pop(b - 1))
        emit_stage2(BPC - 1, heads.pop(BPC - 1), *xbs.pop(BPC - 1),
                    last=True)

    if not nc.is_finalized():
        nc.finalize()
    return nc


def _plan_slots(c1, c2):
    """Partition the B batches into BPC groups of NCORES, minimizing
    sum over groups of (max c1 + max c2).  Returns (slots, assign) where
    assign[core][slot] = original batch index."""
    order = np.argsort(-(c1 + c2), kind="stable")
    groups = [list(order[k * NCORES : (k + 1) * NCORES]) for k in range(BPC)]

    def gcost(g):
        return max(c1[i] for i in g) + max(c2[i] for i in g)

    # local search: swap members between groups while it helps
    improved = True
    it = 0
    while improved and it < 200:
        improved = False
        it += 1
        for ga in range(BPC):
            for gb in range(ga + 1, BPC):
                base = gcost(groups[ga]) + gcost(groups[gb])
                for ia in range(NCORES):
                    for ib in range(NCORES):
                        groups[ga][ia], groups[gb][ib] = (
                            groups[gb][ib], groups[ga][ia])
                        new = gcost(groups[ga]) + gcost(groups[gb])
                        if new < base:
                            base = new
                            improved = True
                        else:
                            groups[ga][ia], groups[gb][ib] = (
                                groups[gb][ib], groups[ga][ia])
    slots = tuple(
        (int(max(c1[i] for i in g)), int(max(c2[i] for i in g)))
        for g in groups)
    assign = [[groups[k][core] for k in range(BPC)]
              for core in range(NCORES)]
    return slots, assign


def kernel(x1_bar, seq_lengths1, x2_bar, seq_lengths2):
    x1_bar = np.ascontiguousarray(x1_bar, dtype=np.float32)
    x2_bar = np.ascontiguousarray(x2_bar, dtype=np.float32)
    sl1 = np.asarray(seq_lengths1).astype(np.int32)
    sl2 = np.asarray(seq_lengths2).astype(np.int32)

    c1 = np.clip((sl1 + 127) // 128, 1, NT).astype(np.int64)
    c2 = np.clip((sl2 + 127) // 128, 1, NT).astype(np.int64)
    slots, assign = _plan_slots(c1, c2)

    xt1f = np.ascontiguousarray(x1_bar.transpose(0, 2, 1))
    xt2f = np.ascontiguousarray(x2_bar.transpose(0, 2, 1))
    xb1f = x1_bar.astype(np.float16)
    xb2f = x2_bar.astype(np.float16)

    ar = np.arange(L, dtype=np.int32)
    pad1 = ar[None, :] >= sl1[:, None]  # [B, L] True on padded i
    pad2 = ar[None, :] >= sl2[:, None]
    m2rowf = np.where(pad2, -SENT, 0.0).astype(np.float16)

    def swz(m, val, idx):
        out = np.where(m[idx], val, 0.0).astype(np.float32)  # [BPC, L]
        return np.ascontiguousarray(
            out.reshape(BPC, NT, 128).transpose(2, 0, 1).reshape(
                128, BPC * NT))

    key = slots
    if key not in _NC_CACHE:
        _NC_CACHE.clear()
        _NC_CACHE[key] = build_nc(slots)
    nc = _NC_CACHE[key]

    in_maps = []
    for core in range(NCORES):
        idx = np.array(assign[core], dtype=np.int64)
        in_maps.append({
            "xt1": np.ascontiguousarray(xt1f[idx]),
            "xt2": np.ascontiguousarray(xt2f[idx]),
            "xb1": np.ascontiguousarray(xb1f[idx]),
            "xb2": np.ascontiguousarray(xb2f[idx]),
            "m2row": np.ascontiguousarray(m2rowf[idx]),
            "m1cs": swz(pad1, -SENT, idx),
            "m1c": swz(pad1, NEG, idx),
        })

    res = run_bass_kernel_spmd(nc, in_maps, core_ids=list(range(NCORES)))

    x1t = np.empty((B, L, H), dtype=np.float32)
    x2t = np.empty((B, L, H), dtype=np.float32)
    for core in range(NCORES):
        r = res.results[core]
        for k in range(BPC):
            bi = assign[core][k]
            x1t[bi] = r["o1"][k].astype(np.float32)
            x2t[bi] = r["o2"][k].astype(np.float32)

    y1 = np.empty((B, L, 4 * H), dtype=np.float32)
    y2 = np.empty((B, L, 4 * H), dtype=np.float32)
    y1[:, :, 0:H] = x1_bar
    y1[:, :, H : 2 * H] = x1t
    y1[:, :, 2 * H : 3 * H] = x1_bar - x1t
    y1[:, :, 3 * H :] = x1_bar * x1t
    y2[:, :, 0:H] = x2_bar
    y2[:, :, H : 2 * H] = x2t
    y2[:, :, 2 * H : 3 * H] = x2_bar - x2t
    y2[:, :, 3 * H :] = x2_bar * x2t
    return y1, y2


# revision 37
# speedup vs baseline: 1.3086x; 1.0768x over previous
"""Trainium2 Bass kernel for nn_LocalInferenceModeling (cross-attention enhance).

Reference computation (per batch b):
    e = x1 @ x2^T                                  [L, L]
    a12 = softmax_j(e + m2[j]);  x1t = a12 @ x2    [L, H]
    a21 = softmax_i(e^T + m1[i]); x2t = a21 @ x1   [L, H]
    y1 = concat([x1, x1t, x1 - x1t, x1 * x1t], -1) [L, 4H]
    y2 = concat([x2, x2t, x2 - x2t, x2 * x2t], -1)

Sharding: batch dim B=32 split across 8 NeuronCores (4 batches/core), no
communication.  The device computes x1_tilde / x2_tilde; the host performs
the final (elementwise) enhance/concat on the exact fp32 inputs.

Device-side design:
  - e is computed once, in fp32 (f32r matmuls at full PE rate), in natural
    [i, j] layout.  A fused DVE tensor_tensor_reduce adds the pad masks
    (bf16-exact sentinel -29952 on padded j columns via a gpsimd partition
    broadcast of the mask row, plus per-partition sentinel on padded i) and
    emits the per-row max in the same pass.
  - p12 = exp(e - rowmax) runs on the Activation engine with a per-partition
    bias, emitting the softmax denominator z1 via accum_out for free.  The
    probabilities are normalized in fp16 (x0.5 DVE cost), transposed on the
    PE at 1 cycle/row (fp16), and contracted against fp16 x2 values.  The
    resulting psum is final (already normalized) and is DMA'd straight from
    PSUM to HBM in fp32.
  - p21 reuses e: column max via gpsimd partition reduces over valid-i tiles
    (the j-sentinel rides along and cancels exactly in fp32 when subtracted),
    exp with a true -1e30 per-partition bias for padded i, z2 via rank-1 PE
    matmuls, normalization folded into the psum->SBUF copies (spread across
    Pool/DVE/Act), fp16 output.
  - Sequence-length sparsity: softmax probabilities of fully-padded 128-row
    chunks are exactly zero, so the stage-2 contractions only run over the
    first C1/C2 chunks.  All 8 cores share one program, so the per-slot
    chunk counts are baked as the max over the cores' batches after a
    host-side assignment that groups batches of similar length; the program
    is rebuilt (and cached) per distinct slot signature.
"""

import sys

import numpy as np

sys.path.insert(0, "/opt/trn_rl_repo")

from contextlib import ExitStack

import concourse.bass as bass
import concourse.bacc as bacc
import concourse.bass_isa as bass_isa
import concourse.mybir as mybir
from concourse import masks
from concourse.bass_utils import run_bass_kernel_spmd
from concourse.tile import TileContext

B, L, H = 32, 512, 1024
NCORES = 8
BPC = B // NCORES  # batches per core
NT = L // 128  # 4 partition tiles per L
HT = H // 128  # 8 partition tiles per H

SENT = np.float32(29952.0)  # bf16-exact sentinel magnitude
NEG = np.float32(-1.0e30)

F32 = mybir.dt.float32
F32R = mybir.dt.float32r
FP16 = mybir.dt.float16

Exp = mybir.ActivationFunctionType.Exp
Copy = mybir.ActivationFunctionType.Copy
Add = mybir.AluOpType.add
Max = mybir.AluOpType.max

_NC_CACHE = {}


def build_nc(slots):
    """slots: tuple of BPC (C1, C2) pairs; C = valid 128-chunk count baked
    into slot k of every core."""
    nc = bacc.Bacc(None, target_bir_lowering=False)
    xt1 = nc.dram_tensor("xt1", [BPC, H, L], FP16, kind="ExternalInput")
    xt2 = nc.dram_tensor("xt2", [BPC, H, L], FP16, kind="ExternalInput")
    xb1 = nc.dram_tensor("xb1", [BPC, L, H], FP16, kind="ExternalInput")
    xb2 = nc.dram_tensor("xb2", [BPC, L, H], FP16, kind="ExternalInput")
    m2row = nc.dram_tensor("m2row", [BPC, L], FP16, kind="ExternalInput")
    m1cs = nc.dram_tensor("m1cs", [128, BPC * NT], F32, kind="ExternalInput")
    m1c = nc.dram_tensor("m1c", [128, BPC * NT], F32, kind="ExternalInput")
    o1 = nc.dram_tensor("o1", [BPC, L, H], FP16, kind="ExternalOutput")
    o2 = nc.dram_tensor("o2", [BPC, L, H], FP16, kind="ExternalOutput")

    qSP, qACT, qPL = nc.sync, nc.scalar, nc.gpsimd

    with TileContext(nc) as tc, ExitStack() as ctx:
        from concourse.tile import add_dep_helper

        const = ctx.enter_context(tc.tile_pool(name="const", bufs=1))
        ident = const.tile([128, 128], FP16)
        masks.make_identity(nc, ident[:])
        onesh = const.tile([128, 1], FP16)
        nc.vector.memset(onesh[:], 1.0)
        onesrow = const.tile([1, 128], FP16)
        nc.vector.memset(onesrow[:], 1.0)
        ones32col = const.tile([128, 1], F32)
        nc.vector.memset(ones32col[:], 1.0)
        ones32 = const.tile([1, 32], F32)
        nc.vector.memset(ones32[:], 1.0)

        xp = ctx.enter_context(tc.tile_pool(name="xp", bufs=2))
        esb = ctx.enter_context(tc.tile_pool(name="esb", bufs=6))
        pmp = ctx.enter_context(tc.tile_pool(name="pmp", bufs=6))
        cmp_ = ctx.enter_context(tc.tile_pool(name="cmp", bufs=2))
        pp = ctx.enter_context(tc.tile_pool(name="pp", bufs=2 * NT))
        ptp = ctx.enter_context(tc.tile_pool(name="ptp", bufs=2 * NT))
        p21p = ctx.enter_context(tc.tile_pool(name="p21p", bufs=2 * NT))
        st = ctx.enter_context(tc.tile_pool(name="st", bufs=6))
        o2p = ctx.enter_context(tc.tile_pool(name="o2p", bufs=8))
        mrp = ctx.enter_context(tc.tile_pool(name="mrp", bufs=1))
        psE = ctx.enter_context(tc.tile_pool(name="psE", bufs=2, space="PSUM"))
        psT = ctx.enter_context(tc.tile_pool(name="psT", bufs=2, space="PSUM"))
        psS = ctx.enter_context(tc.tile_pool(name="psS", bufs=3, space="PSUM"))
        psScr = ctx.enter_context(
            tc.tile_pool(name="psScr", bufs=1, space="PSUM"))
        scratch = psScr.tile([32, 32], F32, name="scratch", tag="scratch")

        gates = {"psE": [], "psT": [], "psS": []}
        touch_cnt = [0]

        def touch(ap):
            # Tiny PE matmul reading `ap` so the PE engine observes the
            # producer's sem tick; real matmuls then carry at most one sync
            # wait. Rotate over scratch columns so touches don't WAW-chain.
            p = min(ap.shape[0], 32)
            f = min(ap.shape[1], 32)
            if ap.dtype == F32R:
                ap = ap.bitcast(F32)
            oc = onesh if ap.dtype == FP16 else ones32col
            col = touch_cnt[0] % 32
            touch_cnt[0] += 1
            with tc.high_priority(offset=200):
                return nc.tensor.matmul(
                    scratch[0:f, col : col + 1], ap[0:p, 0:f], oc[0:p, 0:1],
                    start=True, stop=True)

        def gate(tag, bufs, first_inst):
            # Order the group's first PE write after the touch that observed
            # the release of the slot it reuses (bufs groups back).
            hist = gates[tag]
            k = len(hist)
            if k >= bufs and hist[k - bufs] is not None:
                add_dep_helper(first_inst.ins, hist[k - bufs].ins, sync=False,
                               reason="psum slot gate")
            hist.append(None)
            return k

        def set_gate(tag, k, tinst):
            gates[tag][k] = tinst

        touch(ident)
        nc.tensor.matmul(scratch[0:32, 0:1], ones32[0:1, :], ones32[0:1, 0:1],
                         start=True, stop=True)

        # ---- static mask loads ----
        m2r = mrp.tile([1, BPC * L], FP16, name="m2r", tag="m2r")
        m1cst = mrp.tile([128, BPC * NT], F32, name="m1cst", tag="m1cst")
        m1ct = mrp.tile([128, BPC * NT], F32, name="m1ct", tag="m1ct")
        qACT.dma_start(m2r[:1, :], m2row.rearrange("b l -> (b l)")[None, :])
        qACT.dma_start(m1cst[:], m1cs[:, :])
        qACT.dma_start(m1ct[:], m1c[:, :])

        def load_xt(b, q1=None, q2=None):
            xt1t = xp.tile([128, HT * L], FP16, name="xt1t", tag="xt1t")
            xt2t = xp.tile([128, HT * L], FP16, name="xt2t", tag="xt2t")
            # transposed e operands, in 2-chunk pieces (~0.8us each): the
            # first e matmuls start at first-piece, and short transfers
            # never monopolize an issuing engine
            q1 = q1 or [qSP] * 4
            q2 = q2 or [qSP] * 4
            for qs, t, src in ((q1, xt1t, xt1), (q2, xt2t, xt2)):
                for i in range(4):
                    qs[i].dma_start(
                        t[:, 2 * i * L : 2 * (i + 1) * L].rearrange(
                            "p (c l) -> p c l", c=2),
                        src[b, 2 * i * 128 : 2 * (i + 1) * 128].rearrange(
                            "(c p) l -> p c l", p=128))
            return xt1t, xt2t

        def load_xb(b, q1=None, q2=None):
            C1, C2 = slots[b]
            xb1t = xp.tile([128, NT * H], FP16, name="xb1t", tag="xb1t")
            xb2t = xp.tile([128, NT * H], FP16, name="xb2t", tag="xb2t")
            # natural stage-2 values: only the valid chunks are ever read;
            # per-chunk pieces (~0.8us)
            for q, t, src, C in ((q1 or qACT, xb1t, xb1, C1),
                                 (q2 or qACT, xb2t, xb2, C2)):
                for a in range(C):
                    q.dma_start(
                        t[:, a * H : (a + 1) * H],
                        src[b, a * 128 : (a + 1) * 128])
            return xb1t, xb2t

        def emit_head(b, xt1t, xt2t):
            """e matmuls + masks/rowmax + p12 (fp16) + colmax cm + p21.
            Returns what stage 2 needs."""
            C1, C2 = slots[b]
            touch(xt1t)
            touch(xt2t)
            m2row_b = m2r[0:1, L * b : L * (b + 1)]

            nm4 = st.tile([128, NT], F32, name="nm4", tag="nm4")
            z1 = st.tile([128, 2 * NT], F32, name="z1", tag="z1")
            e_sb = [esb.tile([128, L], F32, name="e_sb", tag="e_sb")
                    for _ in range(NT)]
            p12 = [pp.tile([128, L], FP16, name="p12", tag="p12")
                   for _ in range(NT)]
            pm = [pmp.tile([128, L], F32, name="pm", tag="pm")
                  for _ in range(C1)]
            W2 = C2 * 128
            for a in range(NT):
                pe = psE.tile([128, L], F32, name="psE", tag="psE")
                k = None
                for c in range(HT):
                    inst = nc.tensor.matmul(
                        pe[:],
                        xt1t[:, L * c + 128 * a : L * c + 128 * (a + 1)],
                        xt2t[:, L * c : L * (c + 1)],
                        start=(c == 0),
                        stop=False,
                    )
                    if c == 0:
                        k = gate("psE", 2, inst)
                # j-pad sentinel rank-1 (uniform -SENT on padded j columns)
                nc.tensor.matmul(pe[:], onesrow[0:1, :], m2row_b,
                                 start=False, stop=True)
                # e_sb = e + m2sent + m1sent[i] (per-partition, valid tiles
                # only); negated rowmax via the fused reduce
                sc = (m1cst[:, NT * b + a : NT * b + a + 1]
                      if a < C1 else 0.0)
                nc.vector.tensor_scalar_add(e_sb[a][:], pe[:], sc)
                set_gate("psE", k, touch(e_sb[a]))
                nc.vector.reduce_max(nm4[:, a : a + 1], e_sb[a][:],
                                     axis=mybir.AxisListType.X, negate=True)
                # p12 = exp(e_sb - rowmax) over valid-j chunks; z1 for free
                nc.scalar.activation(
                    p12[a][:, :W2], e_sb[a][:, :W2], Exp,
                    bias=nm4[:, a : a + 1],
                    accum_out=z1[:, a : a + 1])
                # rz1 lands in the upper half of z1
                nc.vector.reciprocal(z1[:, NT + a : NT + a + 1],
                                     z1[:, a : a + 1])
                touch(p12[a])
                if a < C1:
                    nc.gpsimd.partition_all_reduce(
                        pm[a][:], e_sb[a][:], 128, bass_isa.ReduceOp.max)

            # column max over valid i (sentinels cancel on subtraction)
            if C1 == 1:
                cm = pm[0]
            else:
                cm = cmp_.tile([128, L], F32, name="cm", tag="cm")
                nc.vector.tensor_max(cm[:], pm[0][:], pm[1][:])
                for a in range(2, C1):
                    nc.vector.tensor_max(cm[:], cm[:], pm[a][:])

            # p21 = exp(e - colmax) with -1e30 bias on padded i
            p21 = [p21p.tile([128, L], FP16, name="p21", tag="p21")
                   for _ in range(C1)]
            for a in range(C1):
                nc.vector.tensor_sub(e_sb[a][:], e_sb[a][:], cm[:])
                nc.scalar.activation(
                    p21[a][:], e_sb[a][:], Exp,
                    bias=m1ct[:, NT * b + a : NT * b + a + 1])
                touch(p21[a])
            return p12, p21, z1

        cp_engs = [nc.gpsimd, nc.vector, nc.gpsimd]
        cp_i = [0]

        def norm_copy(dst, pt, rz):
            eng = cp_engs[cp_i[0] % 3]
            cp_i[0] += 1
            if eng is nc.scalar:
                eng.activation(dst, pt, Copy, scale=rz)
            else:
                eng.tensor_scalar_mul(dst, pt, rz)

        def emit_stage2(b, head, xb1t, xb2t, last=False):
            C1, C2 = slots[b]
            p12, p21, z1 = head
            touch(xb1t)
            touch(xb2t)

            # stage-2 contraction groups alternate between the psS and psE
            # rings (psE is idle here: e(b+1) has already run), so the
            # norm-copy latency never backpressures the PE
            s2_i = [0]

            def s2_pool():
                s2_i[0] += 1
                return (psS, "psS", 3) if s2_i[0] % 2 else (psE, "psE", 2)

            ysq = []  # deferred output tiles: (a, ys1, ys)

            def x2t_group(a, n, ys):
                pool, tg, nb = s2_pool()
                pt = pool.tile([128, 512], F32, name="s2", tag=tg)
                k = None
                for ai in range(C1):
                    inst = nc.tensor.matmul(
                        pt[:],
                        p21[ai][:, 128 * a : 128 * (a + 1)],
                        xb1t[:, H * ai + 512 * n : H * ai + 512 * (n + 1)],
                        start=(ai == 0), stop=(ai == C1 - 1),
                    )
                    if ai == 0:
                        k = gate(tg, nb, inst)
                norm_copy(ys[:, 512 * n : 512 * (n + 1)], pt[:],
                          rz2[:, a : a + 1])
                set_gate(tg, k, touch(ys[:, 512 * n : 512 * (n + 1)]))

            # transposes of p12 chunks (fp16, exact), interleaved with x2t
            # groups so the psT drain (Pool copy) hides under PE work
            ys_x2 = [o2p.tile([128, H], FP16, name="ys", tag="ys")
                     for _ in range(NT)]
            rz2 = st.tile([128, NT], F32, name="rz2", tag="rz2")
            z2ps = None
            kz2 = None
            p12T = []
            x2q = [(a, n) for a in range(NT) for n in range(2)]
            xi = 0
            for c in range(C2 + 1):
                if c < C2:
                    tp = psT.tile([128, L], FP16, name="psT", tag="psT")
                    k = None
                    for a in range(NT):
                        inst = nc.tensor.matmul(
                            tp[:, 128 * a : 128 * (a + 1)],
                            p12[a][:, 128 * c : 128 * (c + 1)],
                            ident[:], is_transpose=True,
                            start=(a == 0), stop=False,
                        )
                        if a == 0:
                            k = gate("psT", 2, inst)
                    sb = ptp.tile([128, L], FP16, name="p12T", tag="p12T")
                    nc.gpsimd.tensor_copy(sb[:], tp[:])
                    set_gate("psT", k, touch(sb))
                    p12T.append(sb)
                if c == 0:
                    # z2 (rank-1 partition sums of p21) — p21 is long ready
                    # by now, so this never stalls the PE
                    z2ps = psS.tile([128, NT], F32, name="z2ps", tag="psS")
                    for ai in range(C1):
                        for t in range(NT):
                            inst = nc.tensor.matmul(
                                z2ps[:, t : t + 1],
                                p21[ai][:, 128 * t : 128 * (t + 1)],
                                onesh[:], start=(ai == 0 and t == 0),
                                stop=(ai == C1 - 1 and t == NT - 1))
                            if ai == 0 and t == 0:
                                kz2 = gate("psS", 3, inst)
                    nc.vector.reciprocal(rz2[:], z2ps[:])
                    set_gate("psS", kz2, touch(rz2))
                else:
                    # one x2t group between transposes
                    if xi < len(x2q):
                        a, n = x2q[xi]
                        xi += 1
                        x2t_group(a, n, ys_x2[a])

            for a in range(NT):
                ys1 = o2p.tile([128, H], FP16, name="ys1", tag="ys1")
                for n in range(2):
                    pool, tg, nb = s2_pool()
                    pt = pool.tile([128, 512], F32, name="s2", tag=tg)
                    k = None
                    for ci in range(C2):
                        inst = nc.tensor.matmul(
                            pt[:],
                            p12T[ci][:, 128 * a : 128 * (a + 1)],
                            xb2t[:, H * ci + 512 * n : H * ci + 512 * (n + 1)],
                            start=(ci == 0), stop=(ci == C2 - 1),
                        )
                        if ci == 0:
                            k = gate(tg, nb, inst)
                    norm_copy(ys1[:, 512 * n : 512 * (n + 1)], pt[:],
                              z1[:, NT + a : NT + a + 1])
                    set_gate(tg, k, touch(ys1[:, 512 * n : 512 * (n + 1)]))
                if xi < len(x2q):
                    aa, nn = x2q[xi]
                    xi += 1
                    x2t_group(aa, nn, ys_x2[aa])
                if xi < len(x2q):
                    aa, nn = x2q[xi]
                    xi += 1
                    x2t_group(aa, nn, ys_x2[aa])
                rows = slice(128 * a, 128 * (a + 1))
                if last and a == NT - 1:
                    # final tiles: split across all queues to cut the tail
                    bnd = (0, 342, 684, 1024)
                    for qq, q in enumerate((qSP, qACT, qPL)):
                        cs = slice(bnd[qq], bnd[qq + 1])
                        q.dma_start(o1[b, rows, cs], ys1[:, cs])
                        q.dma_start(o2[b, rows, cs], ys_x2[a][:, cs])
                else:
                    qACT.dma_start(o1[b, rows, :], ys1[:])
                    qACT.dma_start(o2[b, rows, :], ys_x2[a][:])

        # ---- software-pipelined batch loop ----
        # PE order: e(0) | e(1) | T/z2/s2(0) | e(2) | T/z2/s2(1) | ...
        # Prologue loads are hand-spread over all three queues (no outputs
        # compete yet); steady state keeps xt on SP (pure DMA queue), xb and
        # outputs on Act, and the Pool queue free for latency-critical
        # copies.
        xts = {0: load_xt(0, [qSP] * 4, [qPL] * 4)}
        xts[1] = load_xt(1, [qSP] * 4, [qPL] * 4)
        xbs = {0: load_xb(0, qSP, qPL)}
        heads = {}
        for b in range(BPC):
            heads[b] = emit_head(b, *xts.pop(b))
            if b + 1 < BPC:
                if b + 1 not in xts:
                    xts[b + 1] = load_xt(b + 1)
                xbs[b + 1] = load_xb(b + 1)
            if b > 0:
                emit_stage2(b - 1, heads.pop(b - 1), *xbs.pop(b - 1))
        emit_stage2(BPC - 1, heads.pop(BPC - 1), *xbs.pop(BPC - 1),
                    last=True)

    if not nc.is_finalized():
        nc.finalize()
    return nc


def _plan_slots(c1, c2):
    """Partition the B batches into BPC groups of NCORES, minimizing
    sum over groups of (max c1 + max c2).  Returns (slots, assign) where
    assign[core][slot] = original batch index."""
    order = np.argsort(-(c1 + c2), kind="stable")
    groups = [list(order[k * NCORES : (k + 1) * NCORES]) for k in range(BPC)]

    def gcost(g):
        return max(c1[i] for i in g) + max(c2[i] for i in g)

    # local search: swap members between groups while it helps
    improved = True
    it = 0
    while improved and it < 200:
        improved = False
        it += 1
        for ga in range(BPC):
            for gb in range(ga + 1, BPC):
                base = gcost(groups[ga]) + gcost(groups[gb])
                for ia in range(NCORES):
                    for ib in range(NCORES):
                        groups[ga][ia], groups[gb][ib] = (
                            groups[gb][ib], groups[ga][ia])
                        new = gcost(groups[ga]) + gcost(groups[gb])
                        if new < base:
                            base = new
                            improved = True
                        else:
                            groups[ga][ia], groups[gb][ib] = (
                                groups[gb][ib], groups[ga][ia])
    slots = tuple(
        (int(max(c1[i] for i in g)), int(max(c2[i] for i in g)))
        for g in groups)
    assign = [[groups[k][core] for k in range(BPC)]
              for core in range(NCORES)]
    return slots, assign


def kernel(x1_bar, seq_lengths1, x2_bar, seq_lengths2):
    x1_bar = np.ascontiguousarray(x1_bar, dtype=np.float32)
    x2_bar = np.ascontiguousarray(x2_bar, dtype=np.float32)
    sl1 = np.asarray(seq_lengths1).astype(np.int32)
    sl2 = np.asarray(seq_lengths2).astype(np.int32)

    c1 = np.clip((sl1 + 127) // 128, 1, NT).astype(np.int64)
    c2 = np.clip((sl2 + 127) // 128, 1, NT).astype(np.int64)
    slots, assign = _plan_slots(c1, c2)

    xt1f = np.ascontiguousarray(x1_bar.transpose(0, 2, 1)).astype(np.float16)
    xt2f = np.ascontiguousarray(x2_bar.transpose(0, 2, 1)).astype(np.float16)
    xb1f = x1_bar.astype(np.float16)
    xb2f = x2_bar.astype(np.float16)

    ar = np.arange(L, dtype=np.int32)
    pad1 = ar[None, :] >= sl1[:, None]  # [B, L] True on padded i
    pad2 = ar[None, :] >= sl2[:, None]
    m2rowf = np.where(pad2, -SENT, 0.0).astype(np.float16)

    def swz(m, val, idx):
        out = np.where(m[idx], val, 0.0).astype(np.float32)  # [BPC, L]
        return np.ascontiguousarray(
            out.reshape(BPC, NT, 128).transpose(2, 0, 1).reshape(
                128, BPC * NT))

    key = slots
    if key not in _NC_CACHE:
        _NC_CACHE.clear()
        _NC_CACHE[key] = build_nc(slots)
    nc = _NC_CACHE[key]

    in_maps = []
    for core in range(NCORES):
        idx = np.array(assign[core], dtype=np.int64)
        in_maps.append({
            "xt1": np.ascontiguousarray(xt1f[idx]),
            "xt2": np.ascontiguousarray(xt2f[idx]),
            "xb1": np.ascontiguousarray(xb1f[idx]),
            "xb2": np.ascontiguousarray(xb2f[idx]),
            "m2row": np.ascontiguousarray(m2rowf[idx]),
            "m1cs": swz(pad1, -SENT, idx),
            "m1c": swz(pad1, NEG, idx),
        })

    res = run_bass_kernel_spmd(nc, in_maps, core_ids=list(range(NCORES)))

    x1t = np.empty((B, L, H), dtype=np.float32)
    x2t = np.empty((B, L, H), dtype=np.float32)
    for core in range(NCORES):
        r = res.results[core]
        for k in range(BPC):
            bi = assign[core][k]
            x1t[bi] = r["o1"][k].astype(np.float32)
            x2t[bi] = r["o2"][k].astype(np.float32)

    y1 = np.empty((B, L, 4 * H), dtype=np.float32)
    y2 = np.empty((B, L, 4 * H), dtype=np.float32)
    y1[:, :, 0:H] = x1_bar
    y1[:, :, H : 2 * H] = x1t
    y1[:, :, 2 * H : 3 * H] = x1_bar - x1t
    y1[:, :, 3 * H :] = x1_bar * x1t
    y2[:, :, 0:H] = x2_bar
    y2[:, :, H : 2 * H] = x2t
    y2[:, :, 2 * H : 3 * H] = x2_bar - x2t
    y2[:, :, 3 * H :] = x2_bar * x2t
    return y1, y2


# revision 40
# speedup vs baseline: 1.3117x; 1.0024x over previous
"""Trainium2 Bass kernel for nn_LocalInferenceModeling (cross-attention enhance).

Reference computation (per batch b):
    e = x1 @ x2^T                                  [L, L]
    a12 = softmax_j(e + m2[j]);  x1t = a12 @ x2    [L, H]
    a21 = softmax_i(e^T + m1[i]); x2t = a21 @ x1   [L, H]
    y1 = concat([x1, x1t, x1 - x1t, x1 * x1t], -1) [L, 4H]
    y2 = concat([x2, x2t, x2 - x2t, x2 * x2t], -1)

Sharding: batch dim B=32 split across 8 NeuronCores (4 batches/core), no
communication.  The device computes x1_tilde / x2_tilde; the host performs
the final (elementwise) enhance/concat on the exact fp32 inputs.

Device-side design:
  - e is computed once, in fp32 (f32r matmuls at full PE rate), in natural
    [i, j] layout.  A fused DVE tensor_tensor_reduce adds the pad masks
    (bf16-exact sentinel -29952 on padded j columns via a gpsimd partition
    broadcast of the mask row, plus per-partition sentinel on padded i) and
    emits the per-row max in the same pass.
  - p12 = exp(e - rowmax) runs on the Activation engine with a per-partition
    bias, emitting the softmax denominator z1 via accum_out for free.  The
    probabilities are normalized in fp16 (x0.5 DVE cost), transposed on the
    PE at 1 cycle/row (fp16), and contracted against fp16 x2 values.  The
    resulting psum is final (already normalized) and is DMA'd straight from
    PSUM to HBM in fp32.
  - p21 reuses e: column max via gpsimd partition reduces over valid-i tiles
    (the j-sentinel rides along and cancels exactly in fp32 when subtracted),
    exp with a true -1e30 per-partition bias for padded i, z2 via rank-1 PE
    matmuls, normalization folded into the psum->SBUF copies (spread across
    Pool/DVE/Act), fp16 output.
  - Sequence-length sparsity: softmax probabilities of fully-padded 128-row
    chunks are exactly zero, so the stage-2 contractions only run over the
    first C1/C2 chunks.  All 8 cores share one program, so the per-slot
    chunk counts are baked as the max over the cores' batches after a
    host-side assignment that groups batches of similar length; the program
    is rebuilt (and cached) per distinct slot signature.
"""

import sys

import numpy as np

sys.path.insert(0, "/opt/trn_rl_repo")

from contextlib import ExitStack

import concourse.bass as bass
import concourse.bacc as bacc
import concourse.bass_isa as bass_isa
import concourse.mybir as mybir
from concourse import masks
from concourse.bass_utils import run_bass_kernel_spmd
from concourse.tile import TileContext

B, L, H = 32, 512, 1024
NCORES = 8
BPC = B // NCORES  # batches per core
NT = L // 128  # 4 partition tiles per L
HT = H // 128  # 8 partition tiles per H

SENT = np.float32(29952.0)  # bf16-exact sentinel magnitude
NEG = np.float32(-1.0e30)

F32 = mybir.dt.float32
F32R = mybir.dt.float32r
FP16 = mybir.dt.float16

Exp = mybir.ActivationFunctionType.Exp
Copy = mybir.ActivationFunctionType.Copy
Add = mybir.AluOpType.add
Max = mybir.AluOpType.max

_NC_CACHE = {}


def build_nc(slots):
    """slots: tuple of BPC (C1, C2) pairs; C = valid 128-chunk count baked
    into slot k of every core."""
    nc = bacc.Bacc(None, target_bir_lowering=False)
    xt1 = nc.dram_tensor("xt1", [BPC, H, L], FP16, kind="ExternalInput")
    xt2 = nc.dram_tensor("xt2", [BPC, H, L], FP16, kind="ExternalInput")
    xb1 = nc.dram_tensor("xb1", [BPC, L, H], FP16, kind="ExternalInput")
    xb2 = nc.dram_tensor("xb2", [BPC, L, H], FP16, kind="ExternalInput")
    m2row = nc.dram_tensor("m2row", [BPC, L], FP16, kind="ExternalInput")
    m1cs = nc.dram_tensor("m1cs", [128, BPC * NT], F32, kind="ExternalInput")
    m1c = nc.dram_tensor("m1c", [128, BPC * NT], F32, kind="ExternalInput")
    o1 = nc.dram_tensor("o1", [BPC, L, H], FP16, kind="ExternalOutput")
    o2 = nc.dram_tensor("o2", [BPC, L, H], FP16, kind="ExternalOutput")

    qSP, qACT, qPL = nc.sync, nc.scalar, nc.gpsimd

    with TileContext(nc) as tc, ExitStack() as ctx:
        from concourse.tile import add_dep_helper

        const = ctx.enter_context(tc.tile_pool(name="const", bufs=1))
        ident = const.tile([128, 128], FP16)
        masks.make_identity(nc, ident[:])
        onesh = const.tile([128, 1], FP16)
        nc.vector.memset(onesh[:], 1.0)
        onesrow = const.tile([1, 128], FP16)
        nc.vector.memset(onesrow[:], 1.0)
        ones32col = const.tile([128, 1], F32)
        nc.vector.memset(ones32col[:], 1.0)
        ones32 = const.tile([1, 32], F32)
        nc.vector.memset(ones32[:], 1.0)

        xp = ctx.enter_context(tc.tile_pool(name="xp", bufs=2))
        esb = ctx.enter_context(tc.tile_pool(name="esb", bufs=6))
        pmp = ctx.enter_context(tc.tile_pool(name="pmp", bufs=6))
        cmp_ = ctx.enter_context(tc.tile_pool(name="cmp", bufs=2))
        pp = ctx.enter_context(tc.tile_pool(name="pp", bufs=2 * NT))
        ptp = ctx.enter_context(tc.tile_pool(name="ptp", bufs=2 * NT))
        p21p = ctx.enter_context(tc.tile_pool(name="p21p", bufs=2 * NT))
        st = ctx.enter_context(tc.tile_pool(name="st", bufs=6))
        o2p = ctx.enter_context(tc.tile_pool(name="o2p", bufs=8))
        mrp = ctx.enter_context(tc.tile_pool(name="mrp", bufs=1))
        psE = ctx.enter_context(tc.tile_pool(name="psE", bufs=2, space="PSUM"))
        psT = ctx.enter_context(tc.tile_pool(name="psT", bufs=2, space="PSUM"))
        psS = ctx.enter_context(tc.tile_pool(name="psS", bufs=3, space="PSUM"))
        psScr = ctx.enter_context(
            tc.tile_pool(name="psScr", bufs=1, space="PSUM"))
        scratch = psScr.tile([32, 32], F32, name="scratch", tag="scratch")

        gates = {"psE": [], "psT": [], "psS": []}
        touch_cnt = [0]

        def touch(ap):
            # Tiny PE matmul reading `ap` so the PE engine observes the
            # producer's sem tick; real matmuls then carry at most one sync
            # wait. Rotate over scratch columns so touches don't WAW-chain.
            p = min(ap.shape[0], 32)
            f = min(ap.shape[1], 32)
            if ap.dtype == F32R:
                ap = ap.bitcast(F32)
            oc = onesh if ap.dtype == FP16 else ones32col
            col = touch_cnt[0] % 32
            touch_cnt[0] += 1
            with tc.high_priority(offset=200):
                return nc.tensor.matmul(
                    scratch[0:f, col : col + 1], ap[0:p, 0:f], oc[0:p, 0:1],
                    start=True, stop=True)

        def gate(tag, bufs, first_inst):
            # Order the group's first PE write after the touch that observed
            # the release of the slot it reuses (bufs groups back).
            hist = gates[tag]
            k = len(hist)
            if k >= bufs and hist[k - bufs] is not None:
                add_dep_helper(first_inst.ins, hist[k - bufs].ins, sync=False,
                               reason="psum slot gate")
            hist.append(None)
            return k

        def set_gate(tag, k, tinst):
            gates[tag][k] = tinst

        touch(ident)
        nc.tensor.matmul(scratch[0:32, 0:1], ones32[0:1, :], ones32[0:1, 0:1],
                         start=True, stop=True)

        # ---- static mask loads ----
        m2r = mrp.tile([1, BPC * L], FP16, name="m2r", tag="m2r")
        m1cst = mrp.tile([128, BPC * NT], F32, name="m1cst", tag="m1cst")
        m1ct = mrp.tile([128, BPC * NT], F32, name="m1ct", tag="m1ct")
        qACT.dma_start(m1cst[:], m1cs[:, :])
        qACT.dma_start(m2r[:1, :], m2row.rearrange("b l -> (b l)")[None, :])
        qACT.dma_start(m1ct[:], m1c[:, :])

        def load_xt(b, q1=None, q2=None):
            xt1t = xp.tile([128, HT * L], FP16, name="xt1t", tag="xt1t")
            xt2t = xp.tile([128, HT * L], FP16, name="xt2t", tag="xt2t")
            # transposed e operands, in 2-chunk pieces (~0.8us each): the
            # first e matmuls start at first-piece, and short transfers
            # never monopolize an issuing engine
            q1 = q1 or [qSP] * 4
            q2 = q2 or [qSP] * 4
            for qs, t, src in ((q1, xt1t, xt1), (q2, xt2t, xt2)):
                for i in range(4):
                    qs[i].dma_start(
                        t[:, 2 * i * L : 2 * (i + 1) * L].rearrange(
                            "p (c l) -> p c l", c=2),
                        src[b, 2 * i * 128 : 2 * (i + 1) * 128].rearrange(
                            "(c p) l -> p c l", p=128))
            return xt1t, xt2t

        def load_xb(b, q1=None, q2=None):
            C1, C2 = slots[b]
            xb1t = xp.tile([128, NT * H], FP16, name="xb1t", tag="xb1t")
            xb2t = xp.tile([128, NT * H], FP16, name="xb2t", tag="xb2t")
            # natural stage-2 values: only the valid chunks are ever read;
            # per-chunk pieces (~0.8us)
            for q, t, src, C in ((q1 or qACT, xb1t, xb1, C1),
                                 (q2 or qACT, xb2t, xb2, C2)):
                for a in range(C):
                    q.dma_start(
                        t[:, a * H : (a + 1) * H],
                        src[b, a * 128 : (a + 1) * 128])
            return xb1t, xb2t

        def emit_head(b, xt1t, xt2t):
            """e matmuls + masks/rowmax + p12 (fp16) + colmax cm + p21.
            Returns what stage 2 needs."""
            C1, C2 = slots[b]
            touch(xt1t)
            touch(xt2t)
            m2row_b = m2r[0:1, L * b : L * (b + 1)]

            nm4 = st.tile([128, NT], F32, name="nm4", tag="nm4")
            z1 = st.tile([128, 2 * NT], F32, name="z1", tag="z1")
            e_sb = [esb.tile([128, L], F32, name="e_sb", tag="e_sb")
                    for _ in range(NT)]
            p12 = [pp.tile([128, L], FP16, name="p12", tag="p12")
                   for _ in range(NT)]
            pm = [pmp.tile([128, L], F32, name="pm", tag="pm")
                  for _ in range(C1)]
            W2 = C2 * 128
            for a in range(NT):
                pe = psE.tile([128, L], F32, name="psE", tag="psE")
                k = None
                for c in range(HT):
                    inst = nc.tensor.matmul(
                        pe[:],
                        xt1t[:, L * c + 128 * a : L * c + 128 * (a + 1)],
                        xt2t[:, L * c : L * (c + 1)],
                        start=(c == 0),
                        stop=False,
                    )
                    if c == 0:
                        k = gate("psE", 2, inst)
                # j-pad sentinel rank-1 (uniform -SENT on padded j columns)
                nc.tensor.matmul(pe[:], onesrow[0:1, :], m2row_b,
                                 start=False, stop=True)
                # e_sb = e + m2sent + m1sent[i] (per-partition, valid tiles
                # only); negated rowmax via the fused reduce
                sc = (m1cst[:, NT * b + a : NT * b + a + 1]
                      if a < C1 else 0.0)
                nc.vector.tensor_scalar_add(e_sb[a][:], pe[:], sc)
                set_gate("psE", k, touch(e_sb[a]))
                nc.vector.reduce_max(nm4[:, a : a + 1], e_sb[a][:],
                                     axis=mybir.AxisListType.X, negate=True)
                # p12 = exp(e_sb - rowmax) over valid-j chunks; z1 for free
                nc.scalar.activation(
                    p12[a][:, :W2], e_sb[a][:, :W2], Exp,
                    bias=nm4[:, a : a + 1],
                    accum_out=z1[:, a : a + 1])
                # rz1 lands in the upper half of z1
                nc.vector.reciprocal(z1[:, NT + a : NT + a + 1],
                                     z1[:, a : a + 1])
                touch(p12[a])
                if a < C1:
                    nc.gpsimd.partition_all_reduce(
                        pm[a][:], e_sb[a][:], 128, bass_isa.ReduceOp.max)

            # column max over valid i (sentinels cancel on subtraction)
            if C1 == 1:
                cm = pm[0]
            else:
                cm = cmp_.tile([128, L], F32, name="cm", tag="cm")
                nc.vector.tensor_max(cm[:], pm[0][:], pm[1][:])
                for a in range(2, C1):
                    nc.vector.tensor_max(cm[:], cm[:], pm[a][:])

            # p21 = exp(e - colmax) with -1e30 bias on padded i
            p21 = [p21p.tile([128, L], FP16, name="p21", tag="p21")
                   for _ in range(C1)]
            for a in range(C1):
                nc.vector.tensor_sub(e_sb[a][:], e_sb[a][:], cm[:])
                nc.scalar.activation(
                    p21[a][:], e_sb[a][:], Exp,
                    bias=m1ct[:, NT * b + a : NT * b + a + 1])
                touch(p21[a])
            return p12, p21, z1

        cp_engs = [nc.gpsimd, nc.vector, nc.gpsimd]
        cp_i = [0]

        def norm_copy(dst, pt, rz):
            eng = cp_engs[cp_i[0] % 3]
            cp_i[0] += 1
            if eng is nc.scalar:
                eng.activation(dst, pt, Copy, scale=rz)
            else:
                eng.tensor_scalar_mul(dst, pt, rz)

        def emit_stage2(b, head, xb1t, xb2t, last=False):
            C1, C2 = slots[b]
            p12, p21, z1 = head
            touch(xb1t)
            touch(xb2t)

            # stage-2 contraction groups alternate between the psS and psE
            # rings (psE is idle here: e(b+1) has already run), so the
            # norm-copy latency never backpressures the PE
            s2_i = [0]

            def s2_pool():
                s2_i[0] += 1
                return (psS, "psS", 3) if s2_i[0] % 2 else (psE, "psE", 2)

            ysq = []  # deferred output tiles: (a, ys1, ys)

            def x2t_group(a, n, ys):
                pool, tg, nb = s2_pool()
                pt = pool.tile([128, 512], F32, name="s2", tag=tg)
                k = None
                for ai in range(C1):
                    inst = nc.tensor.matmul(
                        pt[:],
                        p21[ai][:, 128 * a : 128 * (a + 1)],
                        xb1t[:, H * ai + 512 * n : H * ai + 512 * (n + 1)],
                        start=(ai == 0), stop=(ai == C1 - 1),
                    )
                    if ai == 0:
                        k = gate(tg, nb, inst)
                norm_copy(ys[:, 512 * n : 512 * (n + 1)], pt[:],
                          rz2[:, a : a + 1])
                set_gate(tg, k, touch(ys[:, 512 * n : 512 * (n + 1)]))

            # transposes of p12 chunks (fp16, exact), interleaved with x2t
            # groups so the psT drain (Pool copy) hides under PE work
            ys_x2 = [o2p.tile([128, H], FP16, name="ys", tag="ys")
                     for _ in range(NT)]
            rz2 = st.tile([128, NT], F32, name="rz2", tag="rz2")
            z2ps = None
            kz2 = None
            p12T = []
            x2q = [(a, n) for a in range(NT) for n in range(2)]
            xi = 0
            for c in range(C2 + 1):
                if c < C2:
                    tp = psT.tile([128, L], FP16, name="psT", tag="psT")
                    k = None
                    for a in range(NT):
                        inst = nc.tensor.matmul(
                            tp[:, 128 * a : 128 * (a + 1)],
                            p12[a][:, 128 * c : 128 * (c + 1)],
                            ident[:], is_transpose=True,
                            start=(a == 0), stop=False,
                        )
                        if a == 0:
                            k = gate("psT", 2, inst)
                    sb = ptp.tile([128, L], FP16, name="p12T", tag="p12T")
                    # alternate the psT drain between Pool and DVE so the
                    # copy latency never gates the transpose ring
                    (nc.gpsimd if c % 2 == 0 else nc.vector).tensor_copy(
                        sb[:], tp[:])
                    set_gate("psT", k, touch(sb))
                    p12T.append(sb)
                if c == 0:
                    # z2 (rank-1 partition sums of p21) — p21 is long ready
                    # by now, so this never stalls the PE
                    z2ps = psS.tile([128, NT], F32, name="z2ps", tag="psS")
                    for ai in range(C1):
                        for t in range(NT):
                            inst = nc.tensor.matmul(
                                z2ps[:, t : t + 1],
                                p21[ai][:, 128 * t : 128 * (t + 1)],
                                onesh[:], start=(ai == 0 and t == 0),
                                stop=(ai == C1 - 1 and t == NT - 1))
                            if ai == 0 and t == 0:
                                kz2 = gate("psS", 3, inst)
                    nc.vector.reciprocal(rz2[:], z2ps[:])
                    set_gate("psS", kz2, touch(rz2))
                else:
                    # one x2t group between transposes
                    if xi < len(x2q):
                        a, n = x2q[xi]
                        xi += 1
                        x2t_group(a, n, ys_x2[a])

            for a in range(NT):
                ys1 = o2p.tile([128, H], FP16, name="ys1", tag="ys1")
                for n in range(2):
                    pool, tg, nb = s2_pool()
                    pt = pool.tile([128, 512], F32, name="s2", tag=tg)
                    k = None
                    for ci in range(C2):
                        inst = nc.tensor.matmul(
                            pt[:],
                            p12T[ci][:, 128 * a : 128 * (a + 1)],
                            xb2t[:, H * ci + 512 * n : H * ci + 512 * (n + 1)],
                            start=(ci == 0), stop=(ci == C2 - 1),
                        )
                        if ci == 0:
                            k = gate(tg, nb, inst)
                    norm_copy(ys1[:, 512 * n : 512 * (n + 1)], pt[:],
                              z1[:, NT + a : NT + a + 1])
                    set_gate(tg, k, touch(ys1[:, 512 * n : 512 * (n + 1)]))
                if xi < len(x2q):
                    aa, nn = x2q[xi]
                    xi += 1
                    x2t_group(aa, nn, ys_x2[aa])
                if xi < len(x2q):
                    aa, nn = x2q[xi]
                    xi += 1
                    x2t_group(aa, nn, ys_x2[aa])
                rows = slice(128 * a, 128 * (a + 1))
                if last and a == NT - 1:
                    # final tiles: split across all queues to cut the tail
                    bnd = (0, 342, 684, 1024)
                    for qq, q in enumerate((qSP, qACT, qPL)):
                        cs = slice(bnd[qq], bnd[qq + 1])
                        q.dma_start(o1[b, rows, cs], ys1[:, cs])
                        q.dma_start(o2[b, rows, cs], ys_x2[a][:, cs])
                else:
                    qSP.dma_start(o1[b, rows, :], ys1[:])
                    qSP.dma_start(o2[b, rows, :], ys_x2[a][:])

        # ---- software-pipelined batch loop ----
        # PE order: e(0) | e(1) | T/z2/s2(0) | e(2) | T/z2/s2(1) | ...
        # Prologue loads are hand-spread over all three queues (no outputs
        # compete yet); steady state keeps xt on SP (pure DMA queue), xb and
        # outputs on Act, and the Pool queue free for latency-critical
        # copies.
        xts = {0: load_xt(0, [qSP] * 4, [qPL] * 4)}
        xts[1] = load_xt(1, [qSP] * 4, [qPL] * 4)
        xbs = {0: load_xb(0, qSP, qPL)}
        heads = {}
        for b in range(BPC):
            heads[b] = emit_head(b, *xts.pop(b))
            if b + 1 < BPC:
                if b + 1 not in xts:
                    xts[b + 1] = load_xt(b + 1)
                xbs[b + 1] = load_xb(b + 1)
            if b > 0:
                emit_stage2(b - 1, heads.pop(b - 1), *xbs.pop(b - 1))
        emit_stage2(BPC - 1, heads.pop(BPC - 1), *xbs.pop(BPC - 1),
                    last=True)

    if not nc.is_finalized():
        nc.finalize()
    return nc


def _plan_slots(c1, c2):
    """Partition the B batches into BPC groups of NCORES, minimizing
    sum over groups of (max c1 + max c2).  Returns (slots, assign) where
    assign[core][slot] = original batch index."""
    order = np.argsort(-(c1 + c2), kind="stable")
    groups = [list(order[k * NCORES : (k + 1) * NCORES]) for k in range(BPC)]

    def gcost(g):
        return max(c1[i] for i in g) + max(c2[i] for i in g)

    # local search: swap members between groups while it helps
    improved = True
    it = 0
    while improved and it < 200:
        improved = False
        it += 1
        for ga in range(BPC):
            for gb in range(ga + 1, BPC):
                base = gcost(groups[ga]) + gcost(groups[gb])
                for ia in range(NCORES):
                    for ib in range(NCORES):
                        groups[ga][ia], groups[gb][ib] = (
                            groups[gb][ib], groups[ga][ia])
                        new = gcost(groups[ga]) + gcost(groups[gb])
                        if new < base:
                            base = new
                            improved = True
                        else:
                            groups[ga][ia], groups[gb][ib] = (
                                groups[gb][ib], groups[ga][ia])
    slots = tuple(
        (int(max(c1[i] for i in g)), int(max(c2[i] for i in g)))
        for g in groups)
    assign = [[groups[k][core] for k in range(BPC)]
              for core in range(NCORES)]
    return slots, assign


def kernel(x1_bar, seq_lengths1, x2_bar, seq_lengths2):
    x1_bar = np.ascontiguousarray(x1_bar, dtype=np.float32)
    x2_bar = np.ascontiguousarray(x2_bar, dtype=np.float32)
    sl1 = np.asarray(seq_lengths1).astype(np.int32)
    sl2 = np.asarray(seq_lengths2).astype(np.int32)

    c1 = np.clip((sl1 + 127) // 128, 1, NT).astype(np.int64)
    c2 = np.clip((sl2 + 127) // 128, 1, NT).astype(np.int64)
    slots, assign = _plan_slots(c1, c2)

    xt1f = np.ascontiguousarray(x1_bar.transpose(0, 2, 1)).astype(np.float16)
    xt2f = np.ascontiguousarray(x2_bar.transpose(0, 2, 1)).astype(np.float16)
    xb1f = x1_bar.astype(np.float16)
    xb2f = x2_bar.astype(np.float16)

    ar = np.arange(L, dtype=np.int32)
    pad1 = ar[None, :] >= sl1[:, None]  # [B, L] True on padded i
    pad2 = ar[None, :] >= sl2[:, None]
    m2rowf = np.where(pad2, -SENT, 0.0).astype(np.float16)

    def swz(m, val, idx):
        out = np.where(m[idx], val, 0.0).astype(np.float32)  # [BPC, L]
        return np.ascontiguousarray(
            out.reshape(BPC, NT, 128).transpose(2, 0, 1).reshape(
                128, BPC * NT))

    key = slots
    if key not in _NC_CACHE:
        _NC_CACHE.clear()
        _NC_CACHE[key] = build_nc(slots)
    nc = _NC_CACHE[key]

    in_maps = []
    for core in range(NCORES):
        idx = np.array(assign[core], dtype=np.int64)
        in_maps.append({
            "xt1": np.ascontiguousarray(xt1f[idx]),
            "xt2": np.ascontiguousarray(xt2f[idx]),
            "xb1": np.ascontiguousarray(xb1f[idx]),
            "xb2": np.ascontiguousarray(xb2f[idx]),
            "m2row": np.ascontiguousarray(m2rowf[idx]),
            "m1cs": swz(pad1, -SENT, idx),
            "m1c": swz(pad1, NEG, idx),
        })

    res = run_bass_kernel_spmd(nc, in_maps, core_ids=list(range(NCORES)))

    x1t = np.empty((B, L, H), dtype=np.float32)
    x2t = np.empty((B, L, H), dtype=np.float32)
    for core in range(NCORES):
        r = res.results[core]
        for k in range(BPC):
            bi = assign[core][k]
            x1t[bi] = r["o1"][k].astype(np.float32)
            x2t[bi] = r["o2"][k].astype(np.float32)

    y1 = np.empty((B, L, 4 * H), dtype=np.float32)
    y2 = np.empty((B, L, 4 * H), dtype=np.float32)
    y1[:, :, 0:H] = x1_bar
    y1[:, :, H : 2 * H] = x1t
    y1[:, :, 2 * H : 3 * H] = x1_bar - x1t
    y1[:, :, 3 * H :] = x1_bar * x1t
    y2[:, :, 0:H] = x2_bar
    y2[:, :, H : 2 * H] = x2t
    y2[:, :, 2 * H : 3 * H] = x2_bar - x2t
    y2[:, :, 3 * H :] = x2_bar * x2t
    return y1, y2
